# revision 46
# baseline (speedup 1.0000x reference)
"""Distributed Trainium2 (8 NeuronCores) GQA attention kernel.

Problem: B=1, T=2048, D=4096, N=32 q-heads, K=8 kv-heads, H=128 (causal,
RMSNorm on q/k/v with (1+scale) on q/k, RoPE base 10000).

Sharding (tensor parallel over heads, per the hint):
  core c owns q-heads [4c, 4c+4) and kv-head c (GQA group preserved, G=4).
  x is replicated (pre-transposed + fp16 on host). Each core computes its
  heads' projections + norms + RoPE + causal attention; per-head attention
  outputs are AllGathered (fp16) as soon as each head finishes, and each
  core computes the final output projection for its own 512-wide slice of
  D. Host concatenates the 8 [2048, 512] f32 slices -> [1, 2048, 4096].
  No partial sums anywhere.

Pipeline: t is processed in 4 groups of 512. Per group block j:
[attn(j,n) + pair-AllGather + proj(j+1,n) for n=0..3], then oproj(j-1)
for all 4 chunks at the BLOCK END -- TensorE's queue is strict FIFO, so
the agt fetches (which wait on the previous group's AllGathers) must sit
behind a full block (~90us) of independent matmuls. Heads are AllGathered
in PAIRS (8 x 2MB ops instead of 16 x 1MB: halves the ~10us-per-op ncfw
floor on the serialized CC stream). The last group computes heads 2,3
first so the tail's first o-proj pass (slots 2,3) has its gather earliest;
only the final pair's latency is exposed, covered by a two-pass PSUM-
partial tail. Chunk loads run with lookahead 2 (issued one proj after
their buffer slot frees) so they never head-of-line block the in-order
sync DMA queue; ag_in bounce writes go via the scalar (HWDGE) queue for a
fast collective trigger.

Precision: fp16 storage for x/weights/q/k/out (8x finer mantissa than bf16
at the same byte width), bf16 for exp(logits) and v (needs exponent range:
softmax is computed WITHOUT max subtraction -- max logit ~68, e^68 fits in
bf16/f32 range but not fp16). All matmul accumulation is f32 in PSUM, norms
and softmax math in f32. Measured rel_l2 vs the f32 reference: ~2.5e-3.

Layout trick: logits are computed TRANSPOSED, lT[s,t] = kT.T @ qT, so that
exp(lT) is directly the AV-matmul rhs (no [t,s]->[s,t] transposes of the
2048x2048 softmax matrix). Fully-masked 128-wide column strips of each
diagonal logit matmul are skipped. The softmax denominator is accumulated
as pT_sum += exp-tile on the VectorEngine (bf16) and reduced over the
partition dim by ONE ones-vector matmul per (head, group) -- the earlier
one-z-matmul-per-k-chunk cost ~38us of TensorE. 1/Z is partition-broadcast
on GpSimd and folded into the PSUM->SBUF copy of the AV output.
"""

import numpy as np

# ---------------------------------------------------------------- constants
T = 2048          # sequence length
D = 4096          # model dim
H = 128           # head dim
NH = 4            # q heads per core
NHEADS = 32       # total q heads
DC = 32           # d-chunks of 128 (contraction tiles)
TC = 16           # t-chunks of 128
NG = 4            # t-groups of 512 (pipeline granularity)
DSL = 512         # output D slice per core
N_CORES = 8
EPS = 1e-6
ROPE_BASE = 10000.0

_CACHE = {}


# ---------------------------------------------------------------- builder
def _build():
    import concourse.mybir as mybir
    import concourse.tile as tile
    from concourse import bacc
    from concourse.masks import make_identity

    FP16 = mybir.dt.float16
    BF16 = mybir.dt.bfloat16
    F32 = mybir.dt.float32
    Act = mybir.ActivationFunctionType
    Alu = mybir.AluOpType

    nc = bacc.Bacc("TRN2", target_bir_lowering=False, debug=False,
                   num_devices=N_CORES)

    # -------- kernel I/O (per-core shards, preprocessed on host)
    xt_d = nc.dram_tensor("xt", [TC, 128, DC, 128], FP16, kind="ExternalInput")
    wq_d = nc.dram_tensor("wq", [128, DC, NH * 128], FP16, kind="ExternalInput")
    wkv_d = nc.dram_tensor("wkv", [128, DC, 256], FP16, kind="ExternalInput")
    wo_d = nc.dram_tensor("wo", [128, NHEADS, DSL], FP16, kind="ExternalInput")
    cs_d = nc.dram_tensor("csp", [TC, 128, 2 * NH * 64], F32,
                          kind="ExternalInput")
    qsb_d = nc.dram_tensor("qsb", [128, NH * 128], F32, kind="ExternalInput")
    ksb_d = nc.dram_tensor("ksb", [128, 128], F32, kind="ExternalInput")
    maskT_d = nc.dram_tensor("maskt", [128, 128], F32, kind="ExternalInput")
    out_d = nc.dram_tensor("out", [T, DSL], F32, kind="ExternalOutput")

    rg = [list(range(N_CORES))]

    with tile.TileContext(nc) as tc:
        with (
            tc.tile_pool(name="wp", bufs=1) as wp,
            tc.tile_pool(name="xp", bufs=3) as xp,
            tc.tile_pool(name="np_", bufs=2) as np_,
            tc.tile_pool(name="pp", bufs=1) as pp,
            tc.tile_pool(name="op", bufs=2) as op,
            tc.tile_pool(name="ps", bufs=1, space="PSUM") as ps,
            tc.tile_pool(name="dr", bufs=1, space="DRAM") as dr,
        ):
            # -------- resident weights / constants
            # wq/wkv split into pieces so the first projection matmuls only
            # wait on the first 0.5MB; wo is deferred (not needed until the
            # first o-proj, ~1/3 into the kernel).
            wq_sb = wp.tile([128, DC * NH * 128], FP16, tag="wq")
            wq_flat = wq_d.ap().rearrange("p a b -> p (a b)")
            wkv_sb = wp.tile([128, DC * 256], FP16, tag="wkv")
            wkv_flat = wkv_d.ap().rearrange("p a b -> p (a b)")
            wo_sb = wp.tile([128, NHEADS * DSL], FP16, tag="wo")
            qsb_sb = wp.tile([128, NH * 128], F32, tag="qsb")
            ksb_sb = wp.tile([128, 128], F32, tag="ksb")
            maskT_sb = wp.tile([128, 128], F32, tag="maskt")
            ident = wp.tile([128, 128], FP16, tag="ident")
            make_identity(nc, ident[:])
            ones_bf = wp.tile([128, 1], BF16, tag="ones")
            nc.vector.memset(ones_bf[:], 1.0)
            eps_sb = wp.tile([128, 1], F32, tag="eps")
            nc.vector.memset(eps_sb[:], EPS)

            # resident K^T [h, s], V [s, h] (fp16 / bf16), one kv head
            kT_sb = wp.tile([128, T], FP16, tag="kT")
            vf_sb = wp.tile([128, T], BF16, tag="vf")

            def load_chunk(ti):
                """Issue the input DMAs for t-chunk ti (x slab + rope)."""
                xt = xp.tile([128, DC * 128], FP16, tag="xt")
                xt_src = xt_d.ap()[ti].rearrange("p a b -> p (a b)")
                for i in range(4):
                    nc.sync.dma_start(xt[:, i * 1024:(i + 1) * 1024],
                                      xt_src[:, i * 1024:(i + 1) * 1024])
                cs_t = np_.tile([128, 2 * NH * 64], F32, tag="cs", bufs=4)
                nc.sync.dma_start(cs_t[:], cs_d.ap()[ti])
                return xt, cs_t

            def proj_chunk(j, tl, qT, xt, cs_t):
                """Project q/k/v for t-chunk ti, normalize, rope, store."""
                ti = 4 * j + tl

                # kv before q: wkv (1MB) lands long before the full wq
                # (4MB) during the startup window
                kv_ps = ps.tile([128, 256], F32, tag="kvps")
                for dc in range(DC):
                    nc.tensor.matmul(
                        kv_ps[:], lhsT=xt[:, dc * 128:(dc + 1) * 128],
                        rhs=wkv_sb[:, dc * 256:(dc + 1) * 256],
                        start=(dc == 0), stop=(dc == DC - 1))
                q_ps = ps.tile([128, 512], F32, tag="qps")
                for dc in range(DC):
                    nc.tensor.matmul(
                        q_ps[:], lhsT=xt[:, dc * 128:(dc + 1) * 128],
                        rhs=wq_sb[:, dc * 512:(dc + 1) * 512],
                        start=(dc == 0), stop=(dc == DC - 1))

                cos_t = cs_t[:, 0:NH * 64]
                sin_t = cs_t[:, NH * 64:2 * NH * 64]

                # ---- Q: drain PSUM to SBUF fast (frees the bank for the
                # next chunk's accumulation), then norm from the copy
                q_sb = np_.tile([128, 512], F32, tag="q_sb")
                nc.scalar.copy(q_sb[:], q_ps[:])
                sqq = np_.tile([128, NH], F32, tag="sqq")
                scr = np_.tile([128, 128], BF16, tag="scr")
                for n in range(NH):
                    nc.scalar.activation(
                        scr[:], q_sb[:, n * 128:(n + 1) * 128], Act.Square,
                        accum_out=sqq[:, n:n + 1])
                rq = np_.tile([128, NH], F32, tag="rq")
                nc.scalar.activation(rq[:], sqq[:], Act.Sqrt,
                                     scale=1.0 / H, bias=eps_sb[:])
                nc.vector.reciprocal(rq[:], rq[:])

                qa = np_.tile([128, 512], F32, tag="qa")
                nc.vector.tensor_mul(out=qa[:], in0=q_sb[:], in1=qsb_sb[:])
                qf = np_.tile([128, 512], F32, tag="qf")
                t1 = np_.tile([128, 256], F32, tag="t1")
                t2 = np_.tile([128, 256], F32, tag="t2")
                qa3 = qa[:].rearrange("p (n h) -> p n h", n=NH)
                qf3 = qf[:].rearrange("p (n h) -> p n h", n=NH)
                c3 = cos_t.rearrange("p (n h) -> p n h", n=NH)
                s3 = sin_t.rearrange("p (n h) -> p n h", n=NH)
                t13 = t1[:].rearrange("p (n h) -> p n h", n=NH)
                t23 = t2[:].rearrange("p (n h) -> p n h", n=NH)
                x1, x2 = qa3[:, :, 0:64], qa3[:, :, 64:128]
                nc.vector.tensor_mul(out=t13, in0=x1, in1=c3)
                nc.vector.tensor_mul(out=t23, in0=x2, in1=s3)
                nc.vector.tensor_tensor(
                    out=qf3[:, :, 0:64], in0=t13, in1=t23, op=Alu.subtract)
                nc.vector.tensor_mul(out=t13, in0=x2, in1=c3)
                nc.vector.tensor_mul(out=t23, in0=x1, in1=s3)
                nc.vector.tensor_tensor(
                    out=qf3[:, :, 64:128], in0=t13, in1=t23, op=Alu.add)
                qb = np_.tile([128, 512], FP16, tag="qb")
                for n in range(NH):
                    nc.vector.tensor_scalar_mul(
                        out=qb[:, n * 128:(n + 1) * 128],
                        in0=qf[:, n * 128:(n + 1) * 128],
                        scalar1=rq[:, n:n + 1])
                for n in range(NH):
                    tp = ps.tile([128, 128], FP16, tag="tp")
                    nc.tensor.transpose(
                        tp[:], qb[:, n * 128:(n + 1) * 128], ident[:])
                    nc.vector.tensor_copy(
                        out=qT[:, n * 512 + tl * 128: n * 512 + (tl + 1) * 128],
                        in_=tp[:])

                # ---- K: rms stats, (1+ks), rope, fp16, transpose
                kv_sb = np_.tile([128, 256], F32, tag="kv_sb")
                nc.vector.tensor_copy(out=kv_sb[:], in_=kv_ps[:])
                sqk = np_.tile([128, 2], F32, tag="sqk")
                nc.scalar.activation(scr[:], kv_sb[:, 0:128], Act.Square,
                                     accum_out=sqk[:, 0:1])
                nc.scalar.activation(scr[:], kv_sb[:, 128:256], Act.Square,
                                     accum_out=sqk[:, 1:2])
                rk = np_.tile([128, 2], F32, tag="rk")
                nc.scalar.activation(rk[:], sqk[:], Act.Sqrt,
                                     scale=1.0 / H, bias=eps_sb[:])
                nc.vector.reciprocal(rk[:], rk[:])

                ka = np_.tile([128, 128], F32, tag="ka")
                nc.vector.tensor_mul(out=ka[:], in0=kv_sb[:, 0:128],
                                     in1=ksb_sb[:])
                kf = np_.tile([128, 128], F32, tag="kf")
                nc.vector.tensor_mul(out=t1[:, 0:64], in0=ka[:, 0:64],
                                     in1=cos_t[:, 0:64])
                nc.vector.tensor_mul(out=t2[:, 0:64], in0=ka[:, 64:128],
                                     in1=sin_t[:, 0:64])
                nc.vector.tensor_tensor(out=kf[:, 0:64], in0=t1[:, 0:64],
                                        in1=t2[:, 0:64], op=Alu.subtract)
                nc.vector.tensor_mul(out=t1[:, 0:64], in0=ka[:, 64:128],
                                     in1=cos_t[:, 0:64])
                nc.vector.tensor_mul(out=t2[:, 0:64], in0=ka[:, 0:64],
                                     in1=sin_t[:, 0:64])
                nc.vector.tensor_tensor(out=kf[:, 64:128], in0=t1[:, 0:64],
                                        in1=t2[:, 0:64], op=Alu.add)
                kb = np_.tile([128, 128], FP16, tag="kb")
                nc.vector.tensor_scalar_mul(out=kb[:], in0=kf[:],
                                            scalar1=rk[:, 0:1])
                tp = ps.tile([128, 128], FP16, tag="tp")
                nc.tensor.transpose(tp[:], kb[:], ident[:])
                nc.vector.tensor_copy(out=kT_sb[:, ti * 128:(ti + 1) * 128], in_=tp[:])

                # ---- V: rms only, bf16, stays [s, h]
                nc.vector.tensor_scalar_mul(
                    out=vf_sb[:, ti * 128:(ti + 1) * 128],
                    in0=kv_sb[:, 128:256], scalar1=rk[:, 1:2])

            def attn_head(j, n, qT):
                """Causal attention for local head n over t-group j; returns
                the normalized output tile outT [h, 512] (fp16, SBUF)."""
                nk = 4 * (j + 1)  # causal s-chunks for this group
                pT_tiles = []
                # running softmax denominator: pT_sum accumulated on DVE
                # (bf16), reduced over partitions by ONE ones-matmul at the
                # end instead of one per k-chunk (saves TensorE time).
                pT_sum = np_.tile([128, 512], BF16, tag="pTs", name="pTs")
                for k in range(nk):
                    lt = ps.tile([128, 512], F32, tag="lt", bufs=2)
                    dcol0 = k - 4 * j
                    lo = max(dcol0, 0) * 128  # columns left of the diagonal
                    # block are fully masked -- skip computing them
                    nc.tensor.matmul(
                        lt[:, lo:512], lhsT=kT_sb[:, k * 128:(k + 1) * 128],
                        rhs=qT[:, n * 512 + lo:(n + 1) * 512],
                        start=True, stop=True)
                    pT_k = pp.tile([128, 512], BF16, tag=f"pT{k}")
                    dcol = k - 4 * j
                    if dcol >= 0:
                        # diagonal s-chunk: mask in-block upper triangle;
                        # t-chunk columns left of it are fully masked and
                        # never computed, added, or AV-multiplied
                        nc.vector.tensor_add(
                            out=lt[:, dcol * 128:(dcol + 1) * 128],
                            in0=lt[:, dcol * 128:(dcol + 1) * 128],
                            in1=maskT_sb[:])
                        nc.scalar.activation(
                            pT_k[:, dcol * 128:512],
                            lt[:, dcol * 128:512], Act.Exp)
                    else:
                        nc.scalar.activation(pT_k[:], lt[:], Act.Exp)
                    if k == 0:
                        nc.vector.tensor_copy(out=pT_sum[:], in_=pT_k[:])
                    else:
                        lo = max(dcol, 0) * 128
                        nc.vector.tensor_add(out=pT_sum[:, lo:512],
                                             in0=pT_sum[:, lo:512],
                                             in1=pT_k[:, lo:512])
                    pT_tiles.append(pT_k)

                z = ps.tile([1, 512], F32, tag="z")
                nc.tensor.matmul(z[:], lhsT=ones_bf[:], rhs=pT_sum[:],
                                 start=True, stop=True)
                rz = np_.tile([1, 512], F32, tag="rz")
                nc.vector.reciprocal(rz[:], z[:])
                bz = np_.tile([128, 512], F32, tag="bz")
                nc.gpsimd.partition_broadcast(bz[:], rz[:])

                av = ps.tile([128, 512], F32, tag="av")
                for k in range(nk):
                    # k=0 is always full-width (start=True clears the whole
                    # bank); later diagonal chunks only accumulate into the
                    # unmasked column range
                    lo = max(k - 4 * j, 0) * 128
                    nc.tensor.matmul(av[:, lo:512],
                                     lhsT=vf_sb[:, k * 128:(k + 1) * 128],
                                     rhs=pT_tiles[k][:, lo:512],
                                     start=(k == 0), stop=(k == nk - 1))
                outT = op.tile([128, 512], FP16, tag=f"outT{n % 2}")
                nc.vector.tensor_mul(out=outT[:], in0=av[:], in1=bz[:])
                return outT

            def gather_pair(j, p, outT0, outT1):
                """AllGather heads 2p,2p+1 of group j in one op; returns
                per-head views [h, core, t] of the gathered buffer."""
                ag_in = dr.tile([2, 128, 512], FP16, tag=f"agin{j}_{p}")
                # scalar (hwdge) queue: faster trigger path than the sync
                # queue, which is congested with weight/x loads
                nc.scalar.dma_start(ag_in[0], outT0[:])
                nc.scalar.dma_start(ag_in[1], outT1[:])
                ag_out = dr.tile([N_CORES, 2, 128, 512], FP16,
                                 tag=f"agout{j}_{p}", addr_space="Shared")
                nc.gpsimd.collective_compute(
                    "AllGather", Alu.bypass, replica_groups=rg,
                    ins=[ag_in.rearrange("a b c -> (a b c)")],
                    outs=[ag_out.rearrange("a b c d -> (a b c d)")])
                v = ag_out.rearrange("c p h t -> p h c t")
                return v[0], v[1]

            def gather_one(j, n, outT):
                """AllGather a single head (1MB): finer tail granularity."""
                ag_in = dr.tile([128, 512], FP16, tag=f"agsin{j}_{n}")
                nc.scalar.dma_start(ag_in[:], outT[:])
                ag_out = dr.tile([N_CORES, 128, 512], FP16,
                                 tag=f"agsout{j}_{n}", addr_space="Shared")
                nc.gpsimd.collective_compute(
                    "AllGather", Alu.bypass, replica_groups=rg,
                    ins=[ag_in.rearrange("a b -> (a b)")],
                    outs=[ag_out.rearrange("a b c -> (a b c)")])
                return ag_out.rearrange("c h t -> h c t")

            def oproj_slots(ti, ags, slots, o_ps, start, stop):
                for slot in slots:
                    agt = op.tile([128, N_CORES * 128], FP16, tag="agt",
                                  bufs=3, name="agt")
                    nc.sync.dma_start(
                        agt[:].rearrange("p (a b) -> p a b", a=N_CORES),
                        ags[slot][:, :, (ti % 4) * 128:(ti % 4 + 1) * 128])
                    for c8 in range(N_CORES):
                        nhead = 4 * c8 + slot
                        nc.tensor.matmul(
                            o_ps[:],
                            lhsT=agt[:, c8 * 128:(c8 + 1) * 128],
                            rhs=wo_sb[:, nhead * 512:(nhead + 1) * 512],
                            start=(start and slot == slots[0] and c8 == 0),
                            stop=(stop and slot == slots[-1]
                                  and c8 == N_CORES - 1))

            def oproj_chunk(j, tl, ags):
                """Output projection (all 32 global heads -> local D slice)
                for t-chunk tl of group j. ags[n][c] holds core c's
                local head n = global head 4c+n."""
                ti = 4 * j + tl
                o_ps = ps.tile([128, 512], F32, tag="ops")
                oproj_slots(ti, ags, [0, 1, 2, 3], o_ps, True, True)
                o_sb = op.tile([128, 512], F32, tag="osb")
                nc.scalar.copy(o_sb[:], o_ps[:])
                nc.sync.dma_start(
                    out_d.ap()[ti * 128:(ti + 1) * 128, :], o_sb[:])

            def oproj_tail(j, ags):
                """Last group: run slots 2-3 (whose pair AllGather was
                issued FIRST in the last group) for all 4 chunks, then
                slots 0-1 as a second PSUM pass merged with a DVE add, so
                the later pair's AllGather latency is covered."""
                partials = []
                for tl in range(4):
                    o_ps = ps.tile([128, 512], F32, tag="ops")
                    oproj_slots(4 * j + tl, ags, [2, 3], o_ps, True, True)
                    o_sb = op.tile([128, 512], F32, tag="osbp", bufs=4,
                                   name="osbp")
                    nc.scalar.copy(o_sb[:], o_ps[:])
                    partials.append(o_sb)
                for tl in range(4):
                    ti = 4 * j + tl
                    o_ps = ps.tile([128, 512], F32, tag="ops")
                    oproj_slots(ti, ags, [0, 1], o_ps, True, True)
                    o_sb2 = op.tile([128, 512], F32, tag="osb2")
                    nc.vector.tensor_add(out=o_sb2[:], in0=o_ps[:],
                                         in1=partials[tl][:])
                    nc.sync.dma_start(
                        out_d.ap()[ti * 128:(ti + 1) * 128, :], o_sb2[:])

            # -------- software pipeline, interleaved at head granularity:
            # attn(j,n) ; proj(j+1,n) ; oproj(j-1,n) round-robin so no
            # engine queue gets a monolithic phase block.
            chunks = {}
            prev_ags = None
            qT_cur = np_.tile([128, NH * 512], FP16, tag="qT", name="qT")
            # interleaved preload: first x-chunk and first wq pieces lead
            def load_wq(lo, hi):
                for i in range(lo, hi):
                    nc.sync.dma_start(wq_sb[:, i * 1024:(i + 1) * 1024],
                                      wq_flat[:, i * 1024:(i + 1) * 1024])

            chunks[0] = load_chunk(0)
            for i in range(4):
                nc.sync.dma_start(wkv_sb[:, i * 2048:(i + 1) * 2048],
                                  wkv_flat[:, i * 2048:(i + 1) * 2048])
            load_wq(0, 4)
            chunks[1] = load_chunk(1)
            load_wq(4, 8)
            nc.sync.dma_start(qsb_sb[:], qsb_d.ap())
            nc.sync.dma_start(ksb_sb[:], ksb_d.ap())
            nc.sync.dma_start(maskT_sb[:], maskT_d.ap())
            # lookahead-2 chunk loads: each load is issued one proj AFTER
            # the buffer slot it needs was freed, so it never head-of-line
            # blocks the (strictly in-order) DMA queue. The remaining wq
            # pieces interleave with the chunk loads so neither stream
            # starves the other on the serial queue.
            for tl in range(4):
                proj_chunk(0, tl, qT_cur, *chunks.pop(tl))
                chunks[tl + 2] = load_chunk(tl + 2)
                if tl < 2:
                    load_wq(8 + 4 * tl, 12 + 4 * tl)
            wo_flat = wo_d.ap().rearrange("p a b -> p (a b)")
            for j in range(NG):
                qT_next = (np_.tile([128, NH * 512], FP16, tag="qT", name="qT")
                           if j + 1 < NG else None)
                # Per head: attention first (its pair-AllGather triggers
                # early), then o-proj of the previous group and proj of the
                # next group fill TensorE under the attention latencies.
                ags = [None] * NH
                outs = [None] * NH
                if j < NG - 1:
                    for n in range(NH):
                        outs[n] = attn_head(j, n, qT_cur)
                        if n % 2 == 1:
                            ags[n - 1], ags[n] = gather_pair(
                                j, n // 2, outs[n - 1], outs[n])
                        ti = 4 * (j + 1) + n
                        proj_chunk(j + 1, n, qT_next, *chunks.pop(ti))
                        if ti + 2 < TC:
                            chunks[ti + 2] = load_chunk(ti + 2)
                        if j == 0:
                            nc.sync.dma_start(
                                wo_sb[:, n * 4096:(n + 1) * 4096],
                                wo_flat[:, n * 4096:(n + 1) * 4096])
                    # o-proj of the PREVIOUS group at the END of this block:
                    # TensorE is strict FIFO, so the agt fetches (which wait
                    # on the previous group's AllGathers) must sit behind a
                    # full block (~90us) of independent attn+proj matmuls.
                    if prev_ags is not None:
                        for n in range(NH):
                            oproj_chunk(j - 1, n, prev_ags)
                else:
                    # last group: heads 2,3 first so their AllGather (needed
                    # by the tail's FIRST pass) is in flight earliest; the
                    # previous group's o-proj fills the remaining latency.
                    outs[2] = attn_head(j, 2, qT_cur)
                    outs[3] = attn_head(j, 3, qT_cur)
                    ags[2], ags[3] = gather_pair(j, 1, outs[2], outs[3])
                    outs[0] = attn_head(j, 0, qT_cur)
                    outs[1] = attn_head(j, 1, qT_cur)
                    ags[0], ags[1] = gather_pair(j, 0, outs[0], outs[1])
                    for n in range(NH):
                        oproj_chunk(j - 1, n, prev_ags)
                prev_ags = ags
                qT_cur = qT_next
            oproj_tail(NG - 1, prev_ags)

    nc.compile()
    return nc


def _get_nc():
    if "nc" not in _CACHE:
        _CACHE["nc"] = _build()
    return _CACHE["nc"]


# ---------------------------------------------------------------- host prep
def _make_in_maps(x, segment_pos, attn_mask, q_w, kv_w, o_w, q_scale, k_scale):
    x = np.asarray(x, np.float32)
    q_w = np.asarray(q_w, np.float32)
    kv_w = np.asarray(kv_w, np.float32)
    o_w = np.asarray(o_w, np.float32)
    q_scale = np.asarray(q_scale, np.float32)
    k_scale = np.asarray(k_scale, np.float32)
    pos = np.asarray(segment_pos)[0].astype(np.float32)

    x2 = x[0]  # [T, D]
    # xt[ti, p, dc, tl] = x[ti*128+tl, dc*128+p]
    xt = np.ascontiguousarray(
        x2.reshape(TC, 128, DC, 128).transpose(0, 3, 2, 1)).astype(np.float16)

    frac = 2.0 * np.arange(H // 2, dtype=np.float32) / H
    ts_ = (ROPE_BASE ** frac).astype(np.float32)
    sinu = pos[:, None] / ts_[None, :]          # [T, 64]
    csp = np.concatenate([np.tile(np.cos(sinu), (1, NH)),
                          np.tile(np.sin(sinu), (1, NH))],
                         axis=1).astype(np.float32).reshape(
        TC, 128, 2 * NH * 64)

    maskT = np.ascontiguousarray(
        np.asarray(attn_mask, np.float32)[0, :128, :128].T)

    qs_row = np.tile(1.0 + q_scale, NH)                       # [512]
    qsb = np.ascontiguousarray(
        np.broadcast_to(qs_row[None, :], (128, NH * 128))).astype(np.float32)
    ksb = np.ascontiguousarray(
        np.broadcast_to((1.0 + k_scale)[None, :], (128, 128))).astype(
            np.float32)

    in_maps = []
    for c in range(N_CORES):
        qw_c = q_w[NH * c:NH * (c + 1)]           # [4, D, H]
        # wq[p, dc, n*128+h] = qw_c[n, dc*128+p, h]
        wq = np.ascontiguousarray(
            qw_c.transpose(1, 0, 2).reshape(DC, 128, NH * H).transpose(
                1, 0, 2)).astype(np.float16)
        kv_c = kv_w[:, c]                         # [2, D, H]
        wkv = np.ascontiguousarray(
            kv_c.transpose(1, 0, 2).reshape(DC, 128, 2 * H).transpose(
                1, 0, 2)).astype(np.float16)
        # wo[h, n, dsl] = o_w[n, h, c*512 + dsl]
        wo = np.ascontiguousarray(
            o_w[:, :, DSL * c:DSL * (c + 1)].transpose(1, 0, 2)).astype(
                np.float16)
        in_maps.append({
            "xt": xt, "wq": wq, "wkv": wkv, "wo": wo,
            "csp": csp, "qsb": qsb, "ksb": ksb,
            "maskt": maskT,
        })
    return in_maps


def _execute(in_maps, trace=False):
    from concourse import bass_utils
    nc = _get_nc()
    return bass_utils.run_bass_kernel_spmd(
        nc, in_maps, core_ids=list(range(N_CORES)), trace=trace)


# ---------------------------------------------------------------- entry
def kernel(x, segment_pos, attn_mask, q_w, kv_w, o_w, q_scale, k_scale):
    in_maps = _make_in_maps(x, segment_pos, attn_mask, q_w, kv_w, o_w,
                            q_scale, k_scale)
    res = _execute(in_maps, trace=False)
    outs = [np.asarray(res.results[c]["out"]) for c in range(N_CORES)]
    full = np.concatenate(outs, axis=1).astype(np.float32)
    return full[None]



# revision 47
# speedup vs baseline: 1.0075x; 1.0075x over previous
"""Distributed Trainium2 (8 NeuronCores) GQA attention kernel.

Problem: B=1, T=2048, D=4096, N=32 q-heads, K=8 kv-heads, H=128 (causal,
RMSNorm on q/k/v with (1+scale) on q/k, RoPE base 10000).

Sharding (tensor parallel over heads, per the hint):
  core c owns q-heads [4c, 4c+4) and kv-head c (GQA group preserved, G=4).
  x is replicated (pre-transposed + fp16 on host). Each core computes its
  heads' projections + norms + RoPE + causal attention; per-head attention
  outputs are AllGathered (fp16) as soon as each head finishes, and each
  core computes the final output projection for its own 512-wide slice of
  D. Host concatenates the 8 [2048, 512] f32 slices -> [1, 2048, 4096].
  No partial sums anywhere.

Pipeline: t is processed in 4 groups of 512. Per group block j:
[attn(j,n) + pair-AllGather + proj(j+1,n) for n=0..3], then oproj(j-1)
for all 4 chunks at the BLOCK END -- TensorE's queue is strict FIFO, so
the agt fetches (which wait on the previous group's AllGathers) must sit
behind a full block (~90us) of independent matmuls. Heads are AllGathered
in PAIRS (8 x 2MB ops instead of 16 x 1MB: halves the ~10us-per-op ncfw
floor on the serialized CC stream). The last group computes heads 2,3
first so the tail's first o-proj pass (slots 2,3) has its gather earliest;
only the final pair's latency is exposed, covered by a two-pass PSUM-
partial tail. Chunk loads run with lookahead 2 (issued one proj after
their buffer slot frees) so they never head-of-line block the in-order
sync DMA queue; ag_in bounce writes go via the scalar (HWDGE) queue for a
fast collective trigger.

Precision: fp16 storage for x/weights/q/k/out (8x finer mantissa than bf16
at the same byte width), bf16 for exp(logits) and v (needs exponent range:
softmax is computed WITHOUT max subtraction -- max logit ~68, e^68 fits in
bf16/f32 range but not fp16). All matmul accumulation is f32 in PSUM, norms
and softmax math in f32. Measured rel_l2 vs the f32 reference: ~2.5e-3.

Layout trick: logits are computed TRANSPOSED, lT[s,t] = kT.T @ qT, so that
exp(lT) is directly the AV-matmul rhs (no [t,s]->[s,t] transposes of the
2048x2048 softmax matrix). Fully-masked 128-wide column strips of each
diagonal chunk are skipped in the logit matmul, the exp, the denominator
accumulation AND the AV accumulation (the k=0 AV matmul is always full
width, so its start=True clear covers the bank and narrow accumulates
are safe). The softmax denominator is accumulated
as pT_sum += exp-tile on the VectorEngine (bf16) and reduced over the
partition dim by ONE ones-vector matmul per (head, group) -- the earlier
one-z-matmul-per-k-chunk cost ~38us of TensorE. 1/Z is partition-broadcast
on GpSimd and folded into the PSUM->SBUF copy of the AV output.
"""

import numpy as np

# ---------------------------------------------------------------- constants
T = 2048          # sequence length
D = 4096          # model dim
H = 128           # head dim
NH = 4            # q heads per core
NHEADS = 32       # total q heads
DC = 32           # d-chunks of 128 (contraction tiles)
TC = 16           # t-chunks of 128
NG = 4            # t-groups of 512 (pipeline granularity)
DSL = 512         # output D slice per core
N_CORES = 8
EPS = 1e-6
ROPE_BASE = 10000.0

_CACHE = {}


# ---------------------------------------------------------------- builder
def _build():
    import concourse.mybir as mybir
    import concourse.tile as tile
    from concourse import bacc
    from concourse.masks import make_identity

    FP16 = mybir.dt.float16
    BF16 = mybir.dt.bfloat16
    F32 = mybir.dt.float32
    Act = mybir.ActivationFunctionType
    Alu = mybir.AluOpType

    nc = bacc.Bacc("TRN2", target_bir_lowering=False, debug=False,
                   num_devices=N_CORES)

    # -------- kernel I/O (per-core shards, preprocessed on host)
    xt_d = nc.dram_tensor("xt", [TC, 128, DC, 128], FP16, kind="ExternalInput")
    wq_d = nc.dram_tensor("wq", [128, DC, NH * 128], FP16, kind="ExternalInput")
    wkv_d = nc.dram_tensor("wkv", [128, DC, 256], FP16, kind="ExternalInput")
    wo_d = nc.dram_tensor("wo", [128, NHEADS, DSL], FP16, kind="ExternalInput")
    cs_d = nc.dram_tensor("csp", [TC, 128, 2 * NH * 64], F32,
                          kind="ExternalInput")
    qsb_d = nc.dram_tensor("qsb", [128, NH * 128], F32, kind="ExternalInput")
    ksb_d = nc.dram_tensor("ksb", [128, 128], F32, kind="ExternalInput")
    maskT_d = nc.dram_tensor("maskt", [128, 128], F32, kind="ExternalInput")
    out_d = nc.dram_tensor("out", [T, DSL], F32, kind="ExternalOutput")

    rg = [list(range(N_CORES))]

    with tile.TileContext(nc) as tc:
        with (
            tc.tile_pool(name="wp", bufs=1) as wp,
            tc.tile_pool(name="xp", bufs=3) as xp,
            tc.tile_pool(name="np_", bufs=2) as np_,
            tc.tile_pool(name="pp", bufs=1) as pp,
            tc.tile_pool(name="op", bufs=2) as op,
            tc.tile_pool(name="ps", bufs=1, space="PSUM") as ps,
            tc.tile_pool(name="dr", bufs=1, space="DRAM") as dr,
        ):
            # -------- resident weights / constants
            # wq/wkv split into pieces so the first projection matmuls only
            # wait on the first 0.5MB; wo is deferred (not needed until the
            # first o-proj, ~1/3 into the kernel).
            wq_sb = wp.tile([128, DC * NH * 128], FP16, tag="wq")
            wq_flat = wq_d.ap().rearrange("p a b -> p (a b)")
            wkv_sb = wp.tile([128, DC * 256], FP16, tag="wkv")
            wkv_flat = wkv_d.ap().rearrange("p a b -> p (a b)")
            wo_sb = wp.tile([128, NHEADS * DSL], FP16, tag="wo")
            qsb_sb = wp.tile([128, NH * 128], F32, tag="qsb")
            ksb_sb = wp.tile([128, 128], F32, tag="ksb")
            maskT_sb = wp.tile([128, 128], F32, tag="maskt")
            ident = wp.tile([128, 128], FP16, tag="ident")
            make_identity(nc, ident[:])
            ones_bf = wp.tile([128, 1], BF16, tag="ones")
            nc.vector.memset(ones_bf[:], 1.0)
            eps_sb = wp.tile([128, 1], F32, tag="eps")
            nc.vector.memset(eps_sb[:], EPS)

            # resident K^T [h, s], V [s, h] (fp16 / bf16), one kv head
            kT_sb = wp.tile([128, T], FP16, tag="kT")
            vf_sb = wp.tile([128, T], BF16, tag="vf")

            def load_chunk(ti):
                """Issue the input DMAs for t-chunk ti (x slab + rope)."""
                xt = xp.tile([128, DC * 128], FP16, tag="xt")
                xt_src = xt_d.ap()[ti].rearrange("p a b -> p (a b)")
                for i in range(4):
                    nc.sync.dma_start(xt[:, i * 1024:(i + 1) * 1024],
                                      xt_src[:, i * 1024:(i + 1) * 1024])
                cs_t = np_.tile([128, 2 * NH * 64], F32, tag="cs", bufs=4)
                nc.sync.dma_start(cs_t[:], cs_d.ap()[ti])
                return xt, cs_t

            def proj_chunk(j, tl, qT, xt, cs_t):
                """Project q/k/v for t-chunk ti, normalize, rope, store."""
                ti = 4 * j + tl

                # kv before q: wkv (1MB) lands long before the full wq
                # (4MB) during the startup window
                kv_ps = ps.tile([128, 256], F32, tag="kvps")
                for dc in range(DC):
                    nc.tensor.matmul(
                        kv_ps[:], lhsT=xt[:, dc * 128:(dc + 1) * 128],
                        rhs=wkv_sb[:, dc * 256:(dc + 1) * 256],
                        start=(dc == 0), stop=(dc == DC - 1))
                q_ps = ps.tile([128, 512], F32, tag="qps")
                for dc in range(DC):
                    nc.tensor.matmul(
                        q_ps[:], lhsT=xt[:, dc * 128:(dc + 1) * 128],
                        rhs=wq_sb[:, dc * 512:(dc + 1) * 512],
                        start=(dc == 0), stop=(dc == DC - 1))

                cos_t = cs_t[:, 0:NH * 64]
                sin_t = cs_t[:, NH * 64:2 * NH * 64]

                # ---- Q: drain PSUM to SBUF fast (frees the bank for the
                # next chunk's accumulation), then norm from the copy
                q_sb = np_.tile([128, 512], F32, tag="q_sb")
                nc.scalar.copy(q_sb[:], q_ps[:])
                sqq = np_.tile([128, NH], F32, tag="sqq")
                scr = np_.tile([128, 128], BF16, tag="scr")
                for n in range(NH):
                    nc.scalar.activation(
                        scr[:], q_sb[:, n * 128:(n + 1) * 128], Act.Square,
                        accum_out=sqq[:, n:n + 1])
                rq = np_.tile([128, NH], F32, tag="rq")
                nc.scalar.activation(rq[:], sqq[:], Act.Sqrt,
                                     scale=1.0 / H, bias=eps_sb[:])
                nc.vector.reciprocal(rq[:], rq[:])

                qa = np_.tile([128, 512], F32, tag="qa")
                nc.vector.tensor_mul(out=qa[:], in0=q_sb[:], in1=qsb_sb[:])
                qf = np_.tile([128, 512], F32, tag="qf")
                t1 = np_.tile([128, 256], F32, tag="t1")
                t2 = np_.tile([128, 256], F32, tag="t2")
                qa3 = qa[:].rearrange("p (n h) -> p n h", n=NH)
                qf3 = qf[:].rearrange("p (n h) -> p n h", n=NH)
                c3 = cos_t.rearrange("p (n h) -> p n h", n=NH)
                s3 = sin_t.rearrange("p (n h) -> p n h", n=NH)
                t13 = t1[:].rearrange("p (n h) -> p n h", n=NH)
                t23 = t2[:].rearrange("p (n h) -> p n h", n=NH)
                x1, x2 = qa3[:, :, 0:64], qa3[:, :, 64:128]
                nc.vector.tensor_mul(out=t13, in0=x1, in1=c3)
                nc.vector.tensor_mul(out=t23, in0=x2, in1=s3)
                nc.vector.tensor_tensor(
                    out=qf3[:, :, 0:64], in0=t13, in1=t23, op=Alu.subtract)
                nc.vector.tensor_mul(out=t13, in0=x2, in1=c3)
                nc.vector.tensor_mul(out=t23, in0=x1, in1=s3)
                nc.vector.tensor_tensor(
                    out=qf3[:, :, 64:128], in0=t13, in1=t23, op=Alu.add)
                qb = np_.tile([128, 512], FP16, tag="qb")
                for n in range(NH):
                    nc.vector.tensor_scalar_mul(
                        out=qb[:, n * 128:(n + 1) * 128],
                        in0=qf[:, n * 128:(n + 1) * 128],
                        scalar1=rq[:, n:n + 1])
                for n in range(NH):
                    tp = ps.tile([128, 128], FP16, tag="tp")
                    nc.tensor.transpose(
                        tp[:], qb[:, n * 128:(n + 1) * 128], ident[:])
                    nc.vector.tensor_copy(
                        out=qT[:, n * 512 + tl * 128: n * 512 + (tl + 1) * 128],
                        in_=tp[:])

                # ---- K: rms stats, (1+ks), rope, fp16, transpose
                kv_sb = np_.tile([128, 256], F32, tag="kv_sb")
                nc.vector.tensor_copy(out=kv_sb[:], in_=kv_ps[:])
                sqk = np_.tile([128, 2], F32, tag="sqk")
                nc.scalar.activation(scr[:], kv_sb[:, 0:128], Act.Square,
                                     accum_out=sqk[:, 0:1])
                nc.scalar.activation(scr[:], kv_sb[:, 128:256], Act.Square,
                                     accum_out=sqk[:, 1:2])
                rk = np_.tile([128, 2], F32, tag="rk")
                nc.scalar.activation(rk[:], sqk[:], Act.Sqrt,
                                     scale=1.0 / H, bias=eps_sb[:])
                nc.vector.reciprocal(rk[:], rk[:])

                ka = np_.tile([128, 128], F32, tag="ka")
                nc.vector.tensor_mul(out=ka[:], in0=kv_sb[:, 0:128],
                                     in1=ksb_sb[:])
                kf = np_.tile([128, 128], F32, tag="kf")
                nc.vector.tensor_mul(out=t1[:, 0:64], in0=ka[:, 0:64],
                                     in1=cos_t[:, 0:64])
                nc.vector.tensor_mul(out=t2[:, 0:64], in0=ka[:, 64:128],
                                     in1=sin_t[:, 0:64])
                nc.vector.tensor_tensor(out=kf[:, 0:64], in0=t1[:, 0:64],
                                        in1=t2[:, 0:64], op=Alu.subtract)
                nc.vector.tensor_mul(out=t1[:, 0:64], in0=ka[:, 64:128],
                                     in1=cos_t[:, 0:64])
                nc.vector.tensor_mul(out=t2[:, 0:64], in0=ka[:, 0:64],
                                     in1=sin_t[:, 0:64])
                nc.vector.tensor_tensor(out=kf[:, 64:128], in0=t1[:, 0:64],
                                        in1=t2[:, 0:64], op=Alu.add)
                kb = np_.tile([128, 128], FP16, tag="kb")
                nc.vector.tensor_scalar_mul(out=kb[:], in0=kf[:],
                                            scalar1=rk[:, 0:1])
                tp = ps.tile([128, 128], FP16, tag="tp")
                nc.tensor.transpose(tp[:], kb[:], ident[:])
                nc.vector.tensor_copy(out=kT_sb[:, ti * 128:(ti + 1) * 128], in_=tp[:])

                # ---- V: rms only, bf16, stays [s, h]
                nc.vector.tensor_scalar_mul(
                    out=vf_sb[:, ti * 128:(ti + 1) * 128],
                    in0=kv_sb[:, 128:256], scalar1=rk[:, 1:2])

            def attn_head(j, n, qT):
                """Causal attention for local head n over t-group j; returns
                the normalized output tile outT [h, 512] (fp16, SBUF)."""
                nk = 4 * (j + 1)  # causal s-chunks for this group
                pT_tiles = []
                # running softmax denominator: pT_sum accumulated on DVE
                # (bf16), reduced over partitions by ONE ones-matmul at the
                # end instead of one per k-chunk (saves TensorE time).
                pT_sum = np_.tile([128, 512], BF16, tag="pTs", name="pTs")
                for k in range(nk):
                    lt = ps.tile([128, 512], F32, tag="lt", bufs=2)
                    dcol0 = k - 4 * j
                    lo = max(dcol0, 0) * 128  # columns left of the diagonal
                    # block are fully masked -- skip computing them
                    nc.tensor.matmul(
                        lt[:, lo:512], lhsT=kT_sb[:, k * 128:(k + 1) * 128],
                        rhs=qT[:, n * 512 + lo:(n + 1) * 512],
                        start=True, stop=True)
                    pT_k = pp.tile([128, 512], BF16, tag=f"pT{k}")
                    dcol = k - 4 * j
                    if dcol >= 0:
                        # diagonal s-chunk: mask in-block upper triangle;
                        # t-chunk columns left of it are fully masked and
                        # never computed, added, or AV-multiplied
                        nc.vector.tensor_add(
                            out=lt[:, dcol * 128:(dcol + 1) * 128],
                            in0=lt[:, dcol * 128:(dcol + 1) * 128],
                            in1=maskT_sb[:])
                        nc.scalar.activation(
                            pT_k[:, dcol * 128:512],
                            lt[:, dcol * 128:512], Act.Exp)
                    else:
                        nc.scalar.activation(pT_k[:], lt[:], Act.Exp)
                    if k == 0:
                        nc.vector.tensor_copy(out=pT_sum[:], in_=pT_k[:])
                    else:
                        lo = max(dcol, 0) * 128
                        nc.vector.tensor_add(out=pT_sum[:, lo:512],
                                             in0=pT_sum[:, lo:512],
                                             in1=pT_k[:, lo:512])
                    pT_tiles.append(pT_k)

                z = ps.tile([1, 512], F32, tag="z")
                nc.tensor.matmul(z[:], lhsT=ones_bf[:], rhs=pT_sum[:],
                                 start=True, stop=True)
                rz = np_.tile([1, 512], F32, tag="rz")
                nc.vector.reciprocal(rz[:], z[:])
                bz = np_.tile([128, 512], F32, tag="bz")
                nc.gpsimd.partition_broadcast(bz[:], rz[:])

                av = ps.tile([128, 512], F32, tag="av")
                for k in range(nk):
                    # k=0 is always full-width (start=True clears the whole
                    # bank); later diagonal chunks only accumulate into the
                    # unmasked column range
                    lo = max(k - 4 * j, 0) * 128
                    nc.tensor.matmul(av[:, lo:512],
                                     lhsT=vf_sb[:, k * 128:(k + 1) * 128],
                                     rhs=pT_tiles[k][:, lo:512],
                                     start=(k == 0), stop=(k == nk - 1))
                outT = op.tile([128, 512], FP16, tag=f"outT{n % 2}")
                nc.vector.tensor_mul(out=outT[:], in0=av[:], in1=bz[:])
                return outT

            def gather_pair(j, p, outT0, outT1):
                """AllGather heads 2p,2p+1 of group j in one op; returns
                per-head views [h, core, t] of the gathered buffer."""
                ag_in = dr.tile([2, 128, 512], FP16, tag=f"agin{j}_{p}")
                # scalar (hwdge) queue: faster trigger path than the sync
                # queue, which is congested with weight/x loads
                nc.scalar.dma_start(ag_in[0], outT0[:])
                nc.scalar.dma_start(ag_in[1], outT1[:])
                ag_out = dr.tile([N_CORES, 2, 128, 512], FP16,
                                 tag=f"agout{j}_{p}", addr_space="Shared")
                nc.gpsimd.collective_compute(
                    "AllGather", Alu.bypass, replica_groups=rg,
                    ins=[ag_in.rearrange("a b c -> (a b c)")],
                    outs=[ag_out.rearrange("a b c d -> (a b c d)")])
                v = ag_out.rearrange("c p h t -> p h c t")
                return v[0], v[1]

            def gather_one(j, n, outT):
                """AllGather a single head (1MB): finer tail granularity."""
                ag_in = dr.tile([128, 512], FP16, tag=f"agsin{j}_{n}")
                nc.scalar.dma_start(ag_in[:], outT[:])
                ag_out = dr.tile([N_CORES, 128, 512], FP16,
                                 tag=f"agsout{j}_{n}", addr_space="Shared")
                nc.gpsimd.collective_compute(
                    "AllGather", Alu.bypass, replica_groups=rg,
                    ins=[ag_in.rearrange("a b -> (a b)")],
                    outs=[ag_out.rearrange("a b c -> (a b c)")])
                return ag_out.rearrange("c h t -> h c t")

            def oproj_slots(ti, ags, slots, o_ps, start, stop):
                for slot in slots:
                    agt = op.tile([128, N_CORES * 128], FP16, tag="agt",
                                  bufs=3, name="agt")
                    nc.sync.dma_start(
                        agt[:].rearrange("p (a b) -> p a b", a=N_CORES),
                        ags[slot][:, :, (ti % 4) * 128:(ti % 4 + 1) * 128])
                    for c8 in range(N_CORES):
                        nhead = 4 * c8 + slot
                        nc.tensor.matmul(
                            o_ps[:],
                            lhsT=agt[:, c8 * 128:(c8 + 1) * 128],
                            rhs=wo_sb[:, nhead * 512:(nhead + 1) * 512],
                            start=(start and slot == slots[0] and c8 == 0),
                            stop=(stop and slot == slots[-1]
                                  and c8 == N_CORES - 1))

            def oproj_chunk(j, tl, ags):
                """Output projection (all 32 global heads -> local D slice)
                for t-chunk tl of group j. ags[n][c] holds core c's
                local head n = global head 4c+n."""
                ti = 4 * j + tl
                o_ps = ps.tile([128, 512], F32, tag="ops")
                oproj_slots(ti, ags, [0, 1, 2, 3], o_ps, True, True)
                o_sb = op.tile([128, 512], F32, tag="osb")
                nc.scalar.copy(o_sb[:], o_ps[:])
                nc.sync.dma_start(
                    out_d.ap()[ti * 128:(ti + 1) * 128, :], o_sb[:])

            def oproj_tail(j, ags):
                """Last group: run slots 2-3 (whose pair AllGather was
                issued FIRST in the last group) for all 4 chunks, then
                slots 0-1 as a second PSUM pass merged with a DVE add, so
                the later pair's AllGather latency is covered."""
                partials = []
                for tl in range(4):
                    o_ps = ps.tile([128, 512], F32, tag="ops")
                    oproj_slots(4 * j + tl, ags, [2, 3], o_ps, True, True)
                    o_sb = op.tile([128, 512], F32, tag="osbp", bufs=4,
                                   name="osbp")
                    nc.scalar.copy(o_sb[:], o_ps[:])
                    partials.append(o_sb)
                for tl in range(4):
                    ti = 4 * j + tl
                    o_ps = ps.tile([128, 512], F32, tag="ops")
                    oproj_slots(ti, ags, [0, 1], o_ps, True, True)
                    o_sb2 = op.tile([128, 512], F32, tag="osb2")
                    nc.vector.tensor_add(out=o_sb2[:], in0=o_ps[:],
                                         in1=partials[tl][:])
                    nc.sync.dma_start(
                        out_d.ap()[ti * 128:(ti + 1) * 128, :], o_sb2[:])

            # -------- software pipeline, interleaved at head granularity:
            # attn(j,n) ; proj(j+1,n) ; oproj(j-1,n) round-robin so no
            # engine queue gets a monolithic phase block.
            chunks = {}
            prev_ags = None
            qT_cur = np_.tile([128, NH * 512], FP16, tag="qT", name="qT")
            # interleaved preload: first x-chunk and first wq pieces lead
            def load_wq(lo, hi):
                for i in range(lo, hi):
                    nc.sync.dma_start(wq_sb[:, i * 1024:(i + 1) * 1024],
                                      wq_flat[:, i * 1024:(i + 1) * 1024])

            chunks[0] = load_chunk(0)
            for i in range(4):
                nc.sync.dma_start(wkv_sb[:, i * 2048:(i + 1) * 2048],
                                  wkv_flat[:, i * 2048:(i + 1) * 2048])
            load_wq(0, 4)
            chunks[1] = load_chunk(1)
            load_wq(4, 8)
            nc.sync.dma_start(qsb_sb[:], qsb_d.ap())
            nc.sync.dma_start(ksb_sb[:], ksb_d.ap())
            nc.sync.dma_start(maskT_sb[:], maskT_d.ap())
            # lookahead-2 chunk loads: each load is issued one proj AFTER
            # the buffer slot it needs was freed, so it never head-of-line
            # blocks the (strictly in-order) DMA queue. The remaining wq
            # pieces interleave with the chunk loads so neither stream
            # starves the other on the serial queue.
            for tl in range(4):
                proj_chunk(0, tl, qT_cur, *chunks.pop(tl))
                chunks[tl + 2] = load_chunk(tl + 2)
                if tl < 2:
                    load_wq(8 + 4 * tl, 12 + 4 * tl)
            wo_flat = wo_d.ap().rearrange("p a b -> p (a b)")
            for j in range(NG):
                qT_next = (np_.tile([128, NH * 512], FP16, tag="qT", name="qT")
                           if j + 1 < NG else None)
                # Per head: attention first (its pair-AllGather triggers
                # early), then o-proj of the previous group and proj of the
                # next group fill TensorE under the attention latencies.
                ags = [None] * NH
                outs = [None] * NH
                if j < NG - 1:
                    for n in range(NH):
                        outs[n] = attn_head(j, n, qT_cur)
                        if n % 2 == 1:
                            ags[n - 1], ags[n] = gather_pair(
                                j, n // 2, outs[n - 1], outs[n])
                        ti = 4 * (j + 1) + n
                        proj_chunk(j + 1, n, qT_next, *chunks.pop(ti))
                        if ti + 2 < TC:
                            chunks[ti + 2] = load_chunk(ti + 2)
                        if j == 0:
                            nc.sync.dma_start(
                                wo_sb[:, n * 4096:(n + 1) * 4096],
                                wo_flat[:, n * 4096:(n + 1) * 4096])
                    # o-proj of the PREVIOUS group at the END of this block:
                    # TensorE is strict FIFO, so the agt fetches (which wait
                    # on the previous group's AllGathers) must sit behind a
                    # full block (~90us) of independent attn+proj matmuls.
                    if prev_ags is not None:
                        for n in range(NH):
                            oproj_chunk(j - 1, n, prev_ags)
                else:
                    # last group: heads 2,3 first so their AllGather (needed
                    # by the tail's FIRST pass) is in flight earliest; the
                    # previous group's o-proj fills the remaining latency.
                    outs[2] = attn_head(j, 2, qT_cur)
                    outs[3] = attn_head(j, 3, qT_cur)
                    ags[2], ags[3] = gather_pair(j, 1, outs[2], outs[3])
                    outs[0] = attn_head(j, 0, qT_cur)
                    outs[1] = attn_head(j, 1, qT_cur)
                    ags[0], ags[1] = gather_pair(j, 0, outs[0], outs[1])
                    for n in range(NH):
                        oproj_chunk(j - 1, n, prev_ags)
                prev_ags = ags
                qT_cur = qT_next
            oproj_tail(NG - 1, prev_ags)

    nc.compile()
    return nc


def _get_nc():
    if "nc" not in _CACHE:
        _CACHE["nc"] = _build()
    return _CACHE["nc"]


# ---------------------------------------------------------------- host prep
def _make_in_maps(x, segment_pos, attn_mask, q_w, kv_w, o_w, q_scale, k_scale):
    x = np.asarray(x, np.float32)
    q_w = np.asarray(q_w, np.float32)
    kv_w = np.asarray(kv_w, np.float32)
    o_w = np.asarray(o_w, np.float32)
    q_scale = np.asarray(q_scale, np.float32)
    k_scale = np.asarray(k_scale, np.float32)
    pos = np.asarray(segment_pos)[0].astype(np.float32)

    x2 = x[0]  # [T, D]
    # xt[ti, p, dc, tl] = x[ti*128+tl, dc*128+p]
    xt = np.ascontiguousarray(
        x2.reshape(TC, 128, DC, 128).transpose(0, 3, 2, 1)).astype(np.float16)

    frac = 2.0 * np.arange(H // 2, dtype=np.float32) / H
    ts_ = (ROPE_BASE ** frac).astype(np.float32)
    sinu = pos[:, None] / ts_[None, :]          # [T, 64]
    csp = np.concatenate([np.tile(np.cos(sinu), (1, NH)),
                          np.tile(np.sin(sinu), (1, NH))],
                         axis=1).astype(np.float32).reshape(
        TC, 128, 2 * NH * 64)

    maskT = np.ascontiguousarray(
        np.asarray(attn_mask, np.float32)[0, :128, :128].T)

    qs_row = np.tile(1.0 + q_scale, NH)                       # [512]
    qsb = np.ascontiguousarray(
        np.broadcast_to(qs_row[None, :], (128, NH * 128))).astype(np.float32)
    ksb = np.ascontiguousarray(
        np.broadcast_to((1.0 + k_scale)[None, :], (128, 128))).astype(
            np.float32)

    in_maps = []
    for c in range(N_CORES):
        qw_c = q_w[NH * c:NH * (c + 1)]           # [4, D, H]
        # wq[p, dc, n*128+h] = qw_c[n, dc*128+p, h]
        wq = np.ascontiguousarray(
            qw_c.transpose(1, 0, 2).reshape(DC, 128, NH * H).transpose(
                1, 0, 2)).astype(np.float16)
        kv_c = kv_w[:, c]                         # [2, D, H]
        wkv = np.ascontiguousarray(
            kv_c.transpose(1, 0, 2).reshape(DC, 128, 2 * H).transpose(
                1, 0, 2)).astype(np.float16)
        # wo[h, n, dsl] = o_w[n, h, c*512 + dsl]
        wo = np.ascontiguousarray(
            o_w[:, :, DSL * c:DSL * (c + 1)].transpose(1, 0, 2)).astype(
                np.float16)
        in_maps.append({
            "xt": xt, "wq": wq, "wkv": wkv, "wo": wo,
            "csp": csp, "qsb": qsb, "ksb": ksb,
            "maskt": maskT,
        })
    return in_maps


def _execute(in_maps, trace=False):
    from concourse import bass_utils
    nc = _get_nc()
    return bass_utils.run_bass_kernel_spmd(
        nc, in_maps, core_ids=list(range(N_CORES)), trace=trace)


# ---------------------------------------------------------------- entry
def kernel(x, segment_pos, attn_mask, q_w, kv_w, o_w, q_scale, k_scale):
    in_maps = _make_in_maps(x, segment_pos, attn_mask, q_w, kv_w, o_w,
                            q_scale, k_scale)
    res = _execute(in_maps, trace=False)
    outs = [np.asarray(res.results[c]["out"]) for c in range(N_CORES)]
    full = np.concatenate(outs, axis=1).astype(np.float32)
    return full[None]



# revision 52
# speedup vs baseline: 1.0082x; 1.0007x over previous
"""Distributed Trainium2 (8 NeuronCores) GQA attention kernel.

Problem: B=1, T=2048, D=4096, N=32 q-heads, K=8 kv-heads, H=128 (causal,
RMSNorm on q/k/v with (1+scale) on q/k, RoPE base 10000).

Sharding (tensor parallel over heads, per the hint):
  core c owns q-heads [4c, 4c+4) and kv-head c (GQA group preserved, G=4).
  x is replicated (pre-transposed + fp16 on host). Each core computes its
  heads' projections + norms + RoPE + causal attention; per-head attention
  outputs are AllGathered (fp16) as soon as each head finishes, and each
  core computes the final output projection for its own 512-wide slice of
  D. Host concatenates the 8 [2048, 512] f32 slices -> [1, 2048, 4096].
  No partial sums anywhere.

Pipeline: t is processed in 4 groups of 512. Per group block j:
[attn(j,n) + pair-AllGather + proj(j+1,n) for n=0..3], then oproj(j-1)
for all 4 chunks at the BLOCK END -- TensorE's queue is strict FIFO, so
the agt fetches (which wait on the previous group's AllGathers) must sit
behind a full block (~90us) of independent matmuls. Heads are AllGathered
in PAIRS (8 x 2MB ops instead of 16 x 1MB: halves the ~10us-per-op ncfw
floor on the serialized CC stream). The last group computes heads 2,3
first so the tail's first o-proj pass (slots 2,3) has its gather earliest;
only the final pair's latency is exposed, covered by a two-pass PSUM-
partial tail. Chunk loads run with lookahead 2 (issued one proj after
their buffer slot frees) so they never head-of-line block the in-order
sync DMA queue; ag_in bounce writes go via the scalar (HWDGE) queue for a
fast collective trigger.

Precision: fp16 storage for x/weights/q/k/out (8x finer mantissa than bf16
at the same byte width), bf16 for exp(logits) and v (needs exponent range:
softmax is computed WITHOUT max subtraction -- max logit ~68, e^68 fits in
bf16/f32 range but not fp16). All matmul accumulation is f32 in PSUM, norms
and softmax math in f32. Measured rel_l2 vs the f32 reference: ~2.5e-3.

Layout trick: logits are computed TRANSPOSED, lT[s,t] = kT.T @ qT, so that
exp(lT) is directly the AV-matmul rhs (no [t,s]->[s,t] transposes of the
2048x2048 softmax matrix). Fully-masked 128-wide column strips of each
diagonal chunk are skipped in the logit matmul, the exp, the denominator
accumulation AND the AV accumulation (the k=0 AV matmul is always full
width, so its start=True clear covers the bank and narrow accumulates
are safe). The softmax denominator is accumulated
as pT_sum += exp-tile on the VectorEngine (bf16) and reduced over the
partition dim by ONE ones-vector matmul per (head, group) -- the earlier
one-z-matmul-per-k-chunk cost ~38us of TensorE. 1/Z is partition-broadcast
on GpSimd and folded into the PSUM->SBUF copy of the AV output.
"""

import numpy as np

# ---------------------------------------------------------------- constants
T = 2048          # sequence length
D = 4096          # model dim
H = 128           # head dim
NH = 4            # q heads per core
NHEADS = 32       # total q heads
DC = 32           # d-chunks of 128 (contraction tiles)
TC = 16           # t-chunks of 128
NG = 4            # t-groups of 512 (pipeline granularity)
DSL = 512         # output D slice per core
N_CORES = 8
EPS = 1e-6
ROPE_BASE = 10000.0

_CACHE = {}


# ---------------------------------------------------------------- builder
def _build():
    import concourse.mybir as mybir
    import concourse.tile as tile
    from concourse import bacc
    from concourse.masks import make_identity

    FP16 = mybir.dt.float16
    BF16 = mybir.dt.bfloat16
    F32 = mybir.dt.float32
    Act = mybir.ActivationFunctionType
    Alu = mybir.AluOpType

    nc = bacc.Bacc("TRN2", target_bir_lowering=False, debug=False,
                   num_devices=N_CORES)

    # -------- kernel I/O (per-core shards, preprocessed on host)
    xt_d = nc.dram_tensor("xt", [TC, 128, DC, 128], FP16, kind="ExternalInput")
    wq_d = nc.dram_tensor("wq", [128, DC, NH * 128], FP16, kind="ExternalInput")
    wkv_d = nc.dram_tensor("wkv", [128, DC, 256], FP16, kind="ExternalInput")
    wo_d = nc.dram_tensor("wo", [128, NHEADS, DSL], FP16, kind="ExternalInput")
    cs_d = nc.dram_tensor("csp", [TC, 128, 2 * NH * 64], F32,
                          kind="ExternalInput")
    qsb_d = nc.dram_tensor("qsb", [128, NH * 128], F32, kind="ExternalInput")
    ksb_d = nc.dram_tensor("ksb", [128, 128], F32, kind="ExternalInput")
    maskT_d = nc.dram_tensor("maskt", [128, 128], F32, kind="ExternalInput")
    out_d = nc.dram_tensor("out", [T, DSL], F32, kind="ExternalOutput")

    rg = [list(range(N_CORES))]

    with tile.TileContext(nc) as tc:
        with (
            tc.tile_pool(name="wp", bufs=1) as wp,
            tc.tile_pool(name="xp", bufs=3) as xp,
            tc.tile_pool(name="np_", bufs=2) as np_,
            tc.tile_pool(name="pp", bufs=1) as pp,
            tc.tile_pool(name="op", bufs=2) as op,
            tc.tile_pool(name="ps", bufs=1, space="PSUM") as ps,
            tc.tile_pool(name="dr", bufs=1, space="DRAM") as dr,
        ):
            # -------- resident weights / constants
            # wq/wkv split into pieces so the first projection matmuls only
            # wait on the first 0.5MB; wo is deferred (not needed until the
            # first o-proj, ~1/3 into the kernel).
            wq_sb = wp.tile([128, DC * NH * 128], FP16, tag="wq")
            wq_flat = wq_d.ap().rearrange("p a b -> p (a b)")
            wkv_sb = wp.tile([128, DC * 256], FP16, tag="wkv")
            wkv_flat = wkv_d.ap().rearrange("p a b -> p (a b)")
            wo_sb = wp.tile([128, NHEADS * DSL], FP16, tag="wo")
            qsb_sb = wp.tile([128, NH * 128], F32, tag="qsb")
            ksb_sb = wp.tile([128, 128], F32, tag="ksb")
            maskT_sb = wp.tile([128, 128], F32, tag="maskt")
            ident = wp.tile([128, 128], FP16, tag="ident")
            make_identity(nc, ident[:])
            ones_bf = wp.tile([128, 1], BF16, tag="ones")
            nc.vector.memset(ones_bf[:], 1.0)
            eps_sb = wp.tile([128, 1], F32, tag="eps")
            nc.vector.memset(eps_sb[:], EPS)

            # resident K^T [h, s], V [s, h] (fp16 / bf16), one kv head
            kT_sb = wp.tile([128, T], FP16, tag="kT")
            vf_sb = wp.tile([128, T], BF16, tag="vf")

            def load_chunk(ti):
                """Issue the input DMAs for t-chunk ti (x slab + rope)."""
                xt = xp.tile([128, DC * 128], FP16, tag="xt")
                xt_src = xt_d.ap()[ti].rearrange("p a b -> p (a b)")
                for i in range(4):
                    nc.sync.dma_start(xt[:, i * 1024:(i + 1) * 1024],
                                      xt_src[:, i * 1024:(i + 1) * 1024])
                cs_t = np_.tile([128, 2 * NH * 64], F32, tag="cs", bufs=4)
                nc.sync.dma_start(cs_t[:], cs_d.ap()[ti])
                return xt, cs_t

            # Deferred transposes: a chunk's q/k transposes sit in the
            # TensorE FIFO but depend on the chunk's ~4us Scalar/Vector
            # norm+rope chain. Emitting them right after the chunk's own
            # matmuls stalls TensorE every chunk; instead each chunk
            # enqueues a closure that the NEXT chunk's 64 matmuls cover.
            pending_T = []

            def flush_T():
                while pending_T:
                    pending_T.pop(0)()

            def proj_chunk(j, tl, qT, xt, cs_t):
                """Project q/k/v for t-chunk ti, normalize, rope, store."""
                ti = 4 * j + tl

                # kv before q: wkv (1MB) lands long before the full wq
                # (4MB) during the startup window
                kv_ps = ps.tile([128, 256], F32, tag="kvps")
                for dc in range(DC):
                    nc.tensor.matmul(
                        kv_ps[:], lhsT=xt[:, dc * 128:(dc + 1) * 128],
                        rhs=wkv_sb[:, dc * 256:(dc + 1) * 256],
                        start=(dc == 0), stop=(dc == DC - 1))
                q_ps = ps.tile([128, 512], F32, tag="qps")
                for dc in range(DC):
                    nc.tensor.matmul(
                        q_ps[:], lhsT=xt[:, dc * 128:(dc + 1) * 128],
                        rhs=wq_sb[:, dc * 512:(dc + 1) * 512],
                        start=(dc == 0), stop=(dc == DC - 1))
                flush_T()  # previous chunk's transposes, now covered

                cos_t = cs_t[:, 0:NH * 64]
                sin_t = cs_t[:, NH * 64:2 * NH * 64]

                # ---- Q: drain PSUM to SBUF fast (frees the bank for the
                # next chunk's accumulation), then norm from the copy
                q_sb = np_.tile([128, 512], F32, tag="q_sb")
                nc.scalar.copy(q_sb[:], q_ps[:])
                sqq = np_.tile([128, NH], F32, tag="sqq")
                scr = np_.tile([128, 128], BF16, tag="scr")
                for n in range(NH):
                    nc.scalar.activation(
                        scr[:], q_sb[:, n * 128:(n + 1) * 128], Act.Square,
                        accum_out=sqq[:, n:n + 1])
                rq = np_.tile([128, NH], F32, tag="rq")
                nc.scalar.activation(rq[:], sqq[:], Act.Sqrt,
                                     scale=1.0 / H, bias=eps_sb[:])
                nc.vector.reciprocal(rq[:], rq[:])

                qa = np_.tile([128, 512], F32, tag="qa")
                nc.vector.tensor_mul(out=qa[:], in0=q_sb[:], in1=qsb_sb[:])
                qf = np_.tile([128, 512], F32, tag="qf")
                t1 = np_.tile([128, 256], F32, tag="t1")
                t2 = np_.tile([128, 256], F32, tag="t2")
                qa3 = qa[:].rearrange("p (n h) -> p n h", n=NH)
                qf3 = qf[:].rearrange("p (n h) -> p n h", n=NH)
                c3 = cos_t.rearrange("p (n h) -> p n h", n=NH)
                s3 = sin_t.rearrange("p (n h) -> p n h", n=NH)
                t13 = t1[:].rearrange("p (n h) -> p n h", n=NH)
                t23 = t2[:].rearrange("p (n h) -> p n h", n=NH)
                x1, x2 = qa3[:, :, 0:64], qa3[:, :, 64:128]
                nc.vector.tensor_mul(out=t13, in0=x1, in1=c3)
                nc.vector.tensor_mul(out=t23, in0=x2, in1=s3)
                nc.vector.tensor_tensor(
                    out=qf3[:, :, 0:64], in0=t13, in1=t23, op=Alu.subtract)
                nc.vector.tensor_mul(out=t13, in0=x2, in1=c3)
                nc.vector.tensor_mul(out=t23, in0=x1, in1=s3)
                nc.vector.tensor_tensor(
                    out=qf3[:, :, 64:128], in0=t13, in1=t23, op=Alu.add)
                qb = np_.tile([128, 512], FP16, tag="qb")
                for n in range(NH):
                    nc.vector.tensor_scalar_mul(
                        out=qb[:, n * 128:(n + 1) * 128],
                        in0=qf[:, n * 128:(n + 1) * 128],
                        scalar1=rq[:, n:n + 1])
                def do_q_transposes(qb=qb):
                    for n in range(NH):
                        tp = ps.tile([128, 128], FP16, tag="tp")
                        nc.tensor.transpose(
                            tp[:], qb[:, n * 128:(n + 1) * 128], ident[:])
                        nc.vector.tensor_copy(
                            out=qT[:, n * 512 + tl * 128:
                                   n * 512 + (tl + 1) * 128],
                            in_=tp[:])
                pending_T.append(do_q_transposes)

                # ---- K: rms stats, (1+ks), rope, fp16, transpose
                kv_sb = np_.tile([128, 256], F32, tag="kv_sb")
                nc.vector.tensor_copy(out=kv_sb[:], in_=kv_ps[:])
                sqk = np_.tile([128, 2], F32, tag="sqk")
                nc.scalar.activation(scr[:], kv_sb[:, 0:128], Act.Square,
                                     accum_out=sqk[:, 0:1])
                nc.scalar.activation(scr[:], kv_sb[:, 128:256], Act.Square,
                                     accum_out=sqk[:, 1:2])
                rk = np_.tile([128, 2], F32, tag="rk")
                nc.scalar.activation(rk[:], sqk[:], Act.Sqrt,
                                     scale=1.0 / H, bias=eps_sb[:])
                nc.vector.reciprocal(rk[:], rk[:])

                ka = np_.tile([128, 128], F32, tag="ka")
                nc.vector.tensor_mul(out=ka[:], in0=kv_sb[:, 0:128],
                                     in1=ksb_sb[:])
                kf = np_.tile([128, 128], F32, tag="kf")
                nc.vector.tensor_mul(out=t1[:, 0:64], in0=ka[:, 0:64],
                                     in1=cos_t[:, 0:64])
                nc.vector.tensor_mul(out=t2[:, 0:64], in0=ka[:, 64:128],
                                     in1=sin_t[:, 0:64])
                nc.vector.tensor_tensor(out=kf[:, 0:64], in0=t1[:, 0:64],
                                        in1=t2[:, 0:64], op=Alu.subtract)
                nc.vector.tensor_mul(out=t1[:, 0:64], in0=ka[:, 64:128],
                                     in1=cos_t[:, 0:64])
                nc.vector.tensor_mul(out=t2[:, 0:64], in0=ka[:, 0:64],
                                     in1=sin_t[:, 0:64])
                nc.vector.tensor_tensor(out=kf[:, 64:128], in0=t1[:, 0:64],
                                        in1=t2[:, 0:64], op=Alu.add)
                kb = np_.tile([128, 128], FP16, tag="kb")
                nc.vector.tensor_scalar_mul(out=kb[:], in0=kf[:],
                                            scalar1=rk[:, 0:1])

                def do_k_transpose(kb=kb):
                    tp = ps.tile([128, 128], FP16, tag="tp")
                    nc.tensor.transpose(tp[:], kb[:], ident[:])
                    nc.vector.tensor_copy(
                        out=kT_sb[:, ti * 128:(ti + 1) * 128], in_=tp[:])
                pending_T.append(do_k_transpose)

                # ---- V: rms only, bf16, stays [s, h]
                nc.vector.tensor_scalar_mul(
                    out=vf_sb[:, ti * 128:(ti + 1) * 128],
                    in0=kv_sb[:, 128:256], scalar1=rk[:, 1:2])

            def attn_head(j, n, qT):
                """Causal attention for local head n over t-group j; returns
                the normalized output tile outT [h, 512] (fp16, SBUF)."""
                nk = 4 * (j + 1)  # causal s-chunks for this group
                pT_tiles = []
                # running softmax denominator: pT_sum accumulated on DVE
                # (bf16), reduced over partitions by ONE ones-matmul at the
                # end instead of one per k-chunk (saves TensorE time).
                pT_sum = np_.tile([128, 512], BF16, tag="pTs", name="pTs")
                for k in range(nk):
                    lt = ps.tile([128, 512], F32, tag="lt", bufs=2)
                    dcol0 = k - 4 * j
                    lo = max(dcol0, 0) * 128  # columns left of the diagonal
                    # block are fully masked -- skip computing them
                    nc.tensor.matmul(
                        lt[:, lo:512], lhsT=kT_sb[:, k * 128:(k + 1) * 128],
                        rhs=qT[:, n * 512 + lo:(n + 1) * 512],
                        start=True, stop=True)
                    pT_k = pp.tile([128, 512], BF16, tag=f"pT{k}")
                    dcol = k - 4 * j
                    if dcol >= 0:
                        # diagonal s-chunk: mask in-block upper triangle;
                        # t-chunk columns left of it are fully masked and
                        # never computed, added, or AV-multiplied
                        nc.vector.tensor_add(
                            out=lt[:, dcol * 128:(dcol + 1) * 128],
                            in0=lt[:, dcol * 128:(dcol + 1) * 128],
                            in1=maskT_sb[:])
                        nc.scalar.activation(
                            pT_k[:, dcol * 128:512],
                            lt[:, dcol * 128:512], Act.Exp)
                    else:
                        nc.scalar.activation(pT_k[:], lt[:], Act.Exp)
                    if k == 0:
                        nc.vector.tensor_copy(out=pT_sum[:], in_=pT_k[:])
                    else:
                        lo = max(dcol, 0) * 128
                        nc.vector.tensor_add(out=pT_sum[:, lo:512],
                                             in0=pT_sum[:, lo:512],
                                             in1=pT_k[:, lo:512])
                    pT_tiles.append(pT_k)

                z = ps.tile([1, 512], F32, tag="z")
                nc.tensor.matmul(z[:], lhsT=ones_bf[:], rhs=pT_sum[:],
                                 start=True, stop=True)
                rz = np_.tile([1, 512], F32, tag="rz")
                nc.vector.reciprocal(rz[:], z[:])
                bz = np_.tile([128, 512], F32, tag="bz")
                nc.gpsimd.partition_broadcast(bz[:], rz[:])

                av = ps.tile([128, 512], F32, tag="av")
                for k in range(nk):
                    # k=0 is always full-width (start=True clears the whole
                    # bank); later diagonal chunks only accumulate into the
                    # unmasked column range
                    lo = max(k - 4 * j, 0) * 128
                    nc.tensor.matmul(av[:, lo:512],
                                     lhsT=vf_sb[:, k * 128:(k + 1) * 128],
                                     rhs=pT_tiles[k][:, lo:512],
                                     start=(k == 0), stop=(k == nk - 1))
                outT = op.tile([128, 512], FP16, tag=f"outT{n % 2}")
                nc.vector.tensor_mul(out=outT[:], in0=av[:], in1=bz[:])
                return outT

            def gather_pair(j, p, outT0, outT1):
                """AllGather heads 2p,2p+1 of group j in one op; returns
                per-head views [h, core, t] of the gathered buffer."""
                ag_in = dr.tile([2, 128, 512], FP16, tag=f"agin{j}_{p}")
                # scalar (hwdge) queue: faster trigger path than the sync
                # queue, which is congested with weight/x loads
                nc.scalar.dma_start(ag_in[0], outT0[:])
                nc.scalar.dma_start(ag_in[1], outT1[:])
                ag_out = dr.tile([N_CORES, 2, 128, 512], FP16,
                                 tag=f"agout{j}_{p}", addr_space="Shared")
                nc.gpsimd.collective_compute(
                    "AllGather", Alu.bypass, replica_groups=rg,
                    ins=[ag_in.rearrange("a b c -> (a b c)")],
                    outs=[ag_out.rearrange("a b c d -> (a b c d)")])
                v = ag_out.rearrange("c p h t -> p h c t")
                return v[0], v[1]

            def gather_one(j, n, outT):
                """AllGather a single head (1MB): finer tail granularity."""
                ag_in = dr.tile([128, 512], FP16, tag=f"agsin{j}_{n}")
                nc.scalar.dma_start(ag_in[:], outT[:])
                ag_out = dr.tile([N_CORES, 128, 512], FP16,
                                 tag=f"agsout{j}_{n}", addr_space="Shared")
                nc.gpsimd.collective_compute(
                    "AllGather", Alu.bypass, replica_groups=rg,
                    ins=[ag_in.rearrange("a b -> (a b)")],
                    outs=[ag_out.rearrange("a b c -> (a b c)")])
                return ag_out.rearrange("c h t -> h c t")

            def oproj_slots(ti, ags, slots, o_ps, start, stop):
                for slot in slots:
                    agt = op.tile([128, N_CORES * 128], FP16, tag="agt",
                                  bufs=3, name="agt")
                    nc.sync.dma_start(
                        agt[:].rearrange("p (a b) -> p a b", a=N_CORES),
                        ags[slot][:, :, (ti % 4) * 128:(ti % 4 + 1) * 128])
                    for c8 in range(N_CORES):
                        nhead = 4 * c8 + slot
                        nc.tensor.matmul(
                            o_ps[:],
                            lhsT=agt[:, c8 * 128:(c8 + 1) * 128],
                            rhs=wo_sb[:, nhead * 512:(nhead + 1) * 512],
                            start=(start and slot == slots[0] and c8 == 0),
                            stop=(stop and slot == slots[-1]
                                  and c8 == N_CORES - 1))

            def oproj_chunk(j, tl, ags):
                """Output projection (all 32 global heads -> local D slice)
                for t-chunk tl of group j. ags[n][c] holds core c's
                local head n = global head 4c+n."""
                ti = 4 * j + tl
                o_ps = ps.tile([128, 512], F32, tag="ops")
                oproj_slots(ti, ags, [0, 1, 2, 3], o_ps, True, True)
                o_sb = op.tile([128, 512], F32, tag="osb")
                nc.scalar.copy(o_sb[:], o_ps[:])
                nc.sync.dma_start(
                    out_d.ap()[ti * 128:(ti + 1) * 128, :], o_sb[:])

            def oproj_tail(j, ags):
                """Last group: run slots 2-3 (whose pair AllGather was
                issued FIRST in the last group) for all 4 chunks, then
                slots 0-1 as a second PSUM pass merged with a DVE add, so
                the later pair's AllGather latency is covered."""
                partials = []
                for tl in range(4):
                    o_ps = ps.tile([128, 512], F32, tag="ops")
                    oproj_slots(4 * j + tl, ags, [2, 3], o_ps, True, True)
                    o_sb = op.tile([128, 512], F32, tag="osbp", bufs=4,
                                   name="osbp")
                    nc.scalar.copy(o_sb[:], o_ps[:])
                    partials.append(o_sb)
                for tl in range(4):
                    ti = 4 * j + tl
                    o_ps = ps.tile([128, 512], F32, tag="ops")
                    oproj_slots(ti, ags, [0, 1], o_ps, True, True)
                    o_sb2 = op.tile([128, 512], F32, tag="osb2")
                    nc.vector.tensor_add(out=o_sb2[:], in0=o_ps[:],
                                         in1=partials[tl][:])
                    nc.sync.dma_start(
                        out_d.ap()[ti * 128:(ti + 1) * 128, :], o_sb2[:])

            # -------- software pipeline, interleaved at head granularity:
            # attn(j,n) ; proj(j+1,n) ; oproj(j-1,n) round-robin so no
            # engine queue gets a monolithic phase block.
            chunks = {}
            prev_ags = None
            qT_cur = np_.tile([128, NH * 512], FP16, tag="qT", name="qT")
            # interleaved preload: first x-chunk and first wq pieces lead
            def load_wq(lo, hi):
                for i in range(lo, hi):
                    nc.sync.dma_start(wq_sb[:, i * 1024:(i + 1) * 1024],
                                      wq_flat[:, i * 1024:(i + 1) * 1024])

            chunks[0] = load_chunk(0)
            for i in range(4):
                nc.sync.dma_start(wkv_sb[:, i * 2048:(i + 1) * 2048],
                                  wkv_flat[:, i * 2048:(i + 1) * 2048])
            load_wq(0, 4)
            chunks[1] = load_chunk(1)
            load_wq(4, 8)
            nc.sync.dma_start(qsb_sb[:], qsb_d.ap())
            nc.sync.dma_start(ksb_sb[:], ksb_d.ap())
            nc.sync.dma_start(maskT_sb[:], maskT_d.ap())
            # lookahead-2 chunk loads: each load is issued one proj AFTER
            # the buffer slot it needs was freed, so it never head-of-line
            # blocks the (strictly in-order) DMA queue. The remaining wq
            # pieces interleave with the chunk loads so neither stream
            # starves the other on the serial queue.
            for tl in range(4):
                proj_chunk(0, tl, qT_cur, *chunks.pop(tl))
                chunks[tl + 2] = load_chunk(tl + 2)
                if tl < 2:
                    load_wq(8 + 4 * tl, 12 + 4 * tl)
            flush_T()  # T(0,3): group-0 attention needs qT(0)/kT complete
            wo_flat = wo_d.ap().rearrange("p a b -> p (a b)")
            for j in range(NG):
                qT_next = (np_.tile([128, NH * 512], FP16, tag="qT", name="qT")
                           if j + 1 < NG else None)
                # Per head: attention first (its pair-AllGather triggers
                # early), then o-proj of the previous group and proj of the
                # next group fill TensorE under the attention latencies.
                ags = [None] * NH
                outs = [None] * NH
                if j < NG - 1:
                    for n in range(NH):
                        outs[n] = attn_head(j, n, qT_cur)
                        if n % 2 == 1:
                            ags[n - 1], ags[n] = gather_pair(
                                j, n // 2, outs[n - 1], outs[n])
                        ti = 4 * (j + 1) + n
                        proj_chunk(j + 1, n, qT_next, *chunks.pop(ti))
                        if ti + 2 < TC:
                            chunks[ti + 2] = load_chunk(ti + 2)
                        if j == 0:
                            nc.sync.dma_start(
                                wo_sb[:, n * 4096:(n + 1) * 4096],
                                wo_flat[:, n * 4096:(n + 1) * 4096])
                    # o-proj of the PREVIOUS group at the END of this block:
                    # TensorE is strict FIFO, so the agt fetches (which wait
                    # on the previous group's AllGathers) must sit behind a
                    # full block (~90us) of independent attn+proj matmuls.
                    # The last proj chunk's deferred transposes flush after
                    # the first o-proj chunk (32 matmuls of cover) -- and
                    # always before the NEXT block's attention reads qT/kT.
                    if prev_ags is not None:
                        oproj_chunk(j - 1, 0, prev_ags)
                        flush_T()
                        for n in range(1, NH):
                            oproj_chunk(j - 1, n, prev_ags)
                    else:
                        flush_T()
                else:
                    # last group: heads 2,3 first so their AllGather (needed
                    # by the tail's FIRST pass) is in flight earliest; the
                    # previous group's o-proj fills the remaining latency.
                    outs[2] = attn_head(j, 2, qT_cur)
                    outs[3] = attn_head(j, 3, qT_cur)
                    ags[2], ags[3] = gather_pair(j, 1, outs[2], outs[3])
                    outs[0] = attn_head(j, 0, qT_cur)
                    outs[1] = attn_head(j, 1, qT_cur)
                    ags[0], ags[1] = gather_pair(j, 0, outs[0], outs[1])
                    for n in range(NH):
                        oproj_chunk(j - 1, n, prev_ags)
                prev_ags = ags
                qT_cur = qT_next
            oproj_tail(NG - 1, prev_ags)

    nc.compile()
    return nc


def _get_nc():
    if "nc" not in _CACHE:
        _CACHE["nc"] = _build()
    return _CACHE["nc"]


# ---------------------------------------------------------------- host prep
def _make_in_maps(x, segment_pos, attn_mask, q_w, kv_w, o_w, q_scale, k_scale):
    x = np.asarray(x, np.float32)
    q_w = np.asarray(q_w, np.float32)
    kv_w = np.asarray(kv_w, np.float32)
    o_w = np.asarray(o_w, np.float32)
    q_scale = np.asarray(q_scale, np.float32)
    k_scale = np.asarray(k_scale, np.float32)
    pos = np.asarray(segment_pos)[0].astype(np.float32)

    x2 = x[0]  # [T, D]
    # xt[ti, p, dc, tl] = x[ti*128+tl, dc*128+p]
    xt = np.ascontiguousarray(
        x2.reshape(TC, 128, DC, 128).transpose(0, 3, 2, 1)).astype(np.float16)

    frac = 2.0 * np.arange(H // 2, dtype=np.float32) / H
    ts_ = (ROPE_BASE ** frac).astype(np.float32)
    sinu = pos[:, None] / ts_[None, :]          # [T, 64]
    csp = np.concatenate([np.tile(np.cos(sinu), (1, NH)),
                          np.tile(np.sin(sinu), (1, NH))],
                         axis=1).astype(np.float32).reshape(
        TC, 128, 2 * NH * 64)

    maskT = np.ascontiguousarray(
        np.asarray(attn_mask, np.float32)[0, :128, :128].T)

    qs_row = np.tile(1.0 + q_scale, NH)                       # [512]
    qsb = np.ascontiguousarray(
        np.broadcast_to(qs_row[None, :], (128, NH * 128))).astype(np.float32)
    ksb = np.ascontiguousarray(
        np.broadcast_to((1.0 + k_scale)[None, :], (128, 128))).astype(
            np.float32)

    in_maps = []
    for c in range(N_CORES):
        qw_c = q_w[NH * c:NH * (c + 1)]           # [4, D, H]
        # wq[p, dc, n*128+h] = qw_c[n, dc*128+p, h]
        wq = np.ascontiguousarray(
            qw_c.transpose(1, 0, 2).reshape(DC, 128, NH * H).transpose(
                1, 0, 2)).astype(np.float16)
        kv_c = kv_w[:, c]                         # [2, D, H]
        wkv = np.ascontiguousarray(
            kv_c.transpose(1, 0, 2).reshape(DC, 128, 2 * H).transpose(
                1, 0, 2)).astype(np.float16)
        # wo[h, n, dsl] = o_w[n, h, c*512 + dsl]
        wo = np.ascontiguousarray(
            o_w[:, :, DSL * c:DSL * (c + 1)].transpose(1, 0, 2)).astype(
                np.float16)
        in_maps.append({
            "xt": xt, "wq": wq, "wkv": wkv, "wo": wo,
            "csp": csp, "qsb": qsb, "ksb": ksb,
            "maskt": maskT,
        })
    return in_maps


def _execute(in_maps, trace=False):
    from concourse import bass_utils
    nc = _get_nc()
    return bass_utils.run_bass_kernel_spmd(
        nc, in_maps, core_ids=list(range(N_CORES)), trace=trace)


# ---------------------------------------------------------------- entry
def kernel(x, segment_pos, attn_mask, q_w, kv_w, o_w, q_scale, k_scale):
    in_maps = _make_in_maps(x, segment_pos, attn_mask, q_w, kv_w, o_w,
                            q_scale, k_scale)
    res = _execute(in_maps, trace=False)
    outs = [np.asarray(res.results[c]["out"]) for c in range(N_CORES)]
    full = np.concatenate(outs, axis=1).astype(np.float32)
    return full[None]



# revision 54
# speedup vs baseline: 1.0376x; 1.0292x over previous
"""Distributed Trainium2 (8 NeuronCores) GQA attention kernel.

Problem: B=1, T=2048, D=4096, N=32 q-heads, K=8 kv-heads, H=128 (causal,
RMSNorm on q/k/v with (1+scale) on q/k, RoPE base 10000).

Sharding (tensor parallel over heads, per the hint):
  core c owns q-heads [4c, 4c+4) and kv-head c (GQA group preserved, G=4).
  x is replicated (pre-transposed + fp16 on host). Each core computes its
  heads' projections + norms + RoPE + causal attention; per-head attention
  outputs are AllGathered (fp16) as soon as each head finishes, and each
  core computes the final output projection for its own 512-wide slice of
  D. Host concatenates the 8 [2048, 512] f32 slices -> [1, 2048, 4096].
  No partial sums anywhere.

Pipeline: t is processed in 4 groups of 512. Per group block j:
[attn(j,n) + pair-AllGather + proj(j+1,n) for n=0..3], then oproj(j-1)
for all 4 chunks at the BLOCK END -- TensorE's queue is strict FIFO, so
the agt fetches (which wait on the previous group's AllGathers) must sit
behind a full block (~90us) of independent matmuls. Heads are AllGathered
in PAIRS (8 x 2MB ops instead of 16 x 1MB: halves the ~10us-per-op ncfw
floor on the serialized CC stream). The last group computes heads 2,3
first so the tail's first o-proj pass (slots 2,3) has its gather earliest;
only the final pair's latency is exposed, covered by a two-pass PSUM-
partial tail. Chunk loads run with lookahead 2 (issued one proj after
their buffer slot frees) so they never head-of-line block the in-order
sync DMA queue; ag_in bounce writes go via the scalar (HWDGE) queue for a
fast collective trigger.

Precision: fp16 storage for x/weights/q/k/out (8x finer mantissa than bf16
at the same byte width), bf16 for exp(logits) and v (needs exponent range:
softmax is computed WITHOUT max subtraction -- max logit ~68, e^68 fits in
bf16/f32 range but not fp16). All matmul accumulation is f32 in PSUM, norms
and softmax math in f32. Measured rel_l2 vs the f32 reference: ~2.5e-3.

Layout trick: logits are computed TRANSPOSED, lT[s,t] = kT.T @ qT, so that
exp(lT) is directly the AV-matmul rhs (no [t,s]->[s,t] transposes of the
2048x2048 softmax matrix). Fully-masked 128-wide column strips of each
diagonal chunk are skipped in the logit matmul, the exp, the denominator
accumulation AND the AV accumulation (the k=0 AV matmul is always full
width, so its start=True clear covers the bank and narrow accumulates
are safe). The softmax denominator is accumulated
as pT_sum += exp-tile on the VectorEngine (bf16) and reduced over the
partition dim by ONE ones-vector matmul per (head, group) -- the earlier
one-z-matmul-per-k-chunk cost ~38us of TensorE. 1/Z is partition-broadcast
on GpSimd and folded into the PSUM->SBUF copy of the AV output.
"""

import numpy as np

# ---------------------------------------------------------------- constants
T = 2048          # sequence length
D = 4096          # model dim
H = 128           # head dim
NH = 4            # q heads per core
NHEADS = 32       # total q heads
DC = 32           # d-chunks of 128 (contraction tiles)
TC = 16           # t-chunks of 128
NG = 4            # t-groups of 512 (pipeline granularity)
DSL = 512         # output D slice per core
N_CORES = 8
EPS = 1e-6
ROPE_BASE = 10000.0

_CACHE = {}


# ---------------------------------------------------------------- builder
def _build():
    import concourse.mybir as mybir
    import concourse.tile as tile
    from concourse import bacc
    from concourse.masks import make_identity

    FP16 = mybir.dt.float16
    BF16 = mybir.dt.bfloat16
    F32 = mybir.dt.float32
    Act = mybir.ActivationFunctionType
    Alu = mybir.AluOpType

    nc = bacc.Bacc("TRN2", target_bir_lowering=False, debug=False,
                   num_devices=N_CORES)

    # -------- kernel I/O (per-core shards, preprocessed on host)
    xt_d = nc.dram_tensor("xt", [TC, 128, DC, 128], FP16, kind="ExternalInput")
    wq_d = nc.dram_tensor("wq", [128, DC, NH * 128], FP16, kind="ExternalInput")
    wkv_d = nc.dram_tensor("wkv", [128, DC, 256], FP16, kind="ExternalInput")
    wo_d = nc.dram_tensor("wo", [128, NHEADS, DSL], FP16, kind="ExternalInput")
    cs_d = nc.dram_tensor("csp", [TC, 128, 2 * NH * 64], F32,
                          kind="ExternalInput")
    qsb_d = nc.dram_tensor("qsb", [128, NH * 128], F32, kind="ExternalInput")
    ksb_d = nc.dram_tensor("ksb", [128, 128], F32, kind="ExternalInput")
    maskT_d = nc.dram_tensor("maskt", [128, 128], F32, kind="ExternalInput")
    out_d = nc.dram_tensor("out", [T, DSL], F32, kind="ExternalOutput")

    rg = [list(range(N_CORES))]

    with tile.TileContext(nc) as tc:
        with (
            tc.tile_pool(name="wp", bufs=1) as wp,
            tc.tile_pool(name="xp", bufs=3) as xp,
            tc.tile_pool(name="np_", bufs=2) as np_,
            tc.tile_pool(name="pp", bufs=1) as pp,
            tc.tile_pool(name="op", bufs=2) as op,
            tc.tile_pool(name="ps", bufs=1, space="PSUM") as ps,
            tc.tile_pool(name="dr", bufs=1, space="DRAM") as dr,
        ):
            # -------- resident weights / constants
            # wq/wkv split into pieces so the first projection matmuls only
            # wait on the first 0.5MB; wo is deferred (not needed until the
            # first o-proj, ~1/3 into the kernel).
            wq_sb = wp.tile([128, DC * NH * 128], FP16, tag="wq")
            wq_flat = wq_d.ap().rearrange("p a b -> p (a b)")
            wkv_sb = wp.tile([128, DC * 256], FP16, tag="wkv")
            wkv_flat = wkv_d.ap().rearrange("p a b -> p (a b)")
            wo_sb = wp.tile([128, NHEADS * DSL], FP16, tag="wo")
            qsb_sb = wp.tile([128, NH * 128], F32, tag="qsb")
            ksb_sb = wp.tile([128, 128], F32, tag="ksb")
            maskT_sb = wp.tile([128, 128], F32, tag="maskt")
            ident = wp.tile([128, 128], FP16, tag="ident")
            make_identity(nc, ident[:])
            # full ones MATRIX: the z matmul writes the partition-sum to
            # every output partition, doing the broadcast for free (a
            # [1,512] z + single-lane reciprocal + GpSimd broadcast cost
            # ~5.5us of serial latency per head-group)
            ones_bf = wp.tile([128, 128], BF16, tag="ones")
            nc.vector.memset(ones_bf[:], 1.0)
            eps_sb = wp.tile([128, 1], F32, tag="eps")
            nc.vector.memset(eps_sb[:], EPS)

            # resident K^T [h, s], V [s, h] (fp16 / bf16), one kv head
            kT_sb = wp.tile([128, T], FP16, tag="kT")
            vf_sb = wp.tile([128, T], BF16, tag="vf")

            def load_chunk(ti):
                """Issue the input DMAs for t-chunk ti (x slab + rope)."""
                xt = xp.tile([128, DC * 128], FP16, tag="xt")
                xt_src = xt_d.ap()[ti].rearrange("p a b -> p (a b)")
                for i in range(4):
                    nc.sync.dma_start(xt[:, i * 1024:(i + 1) * 1024],
                                      xt_src[:, i * 1024:(i + 1) * 1024])
                cs_t = np_.tile([128, 2 * NH * 64], F32, tag="cs", bufs=4)
                nc.sync.dma_start(cs_t[:], cs_d.ap()[ti])
                return xt, cs_t

            # Deferred transposes: a chunk's q/k transposes sit in the
            # TensorE FIFO but depend on the chunk's ~4us Scalar/Vector
            # norm+rope chain. Emitting them right after the chunk's own
            # matmuls stalls TensorE every chunk; instead each chunk
            # enqueues a closure that the NEXT chunk's 64 matmuls cover.
            pending_T = []

            def flush_T():
                while pending_T:
                    pending_T.pop(0)()

            def proj_chunk(j, tl, qT, xt, cs_t):
                """Project q/k/v for t-chunk ti, normalize, rope, store."""
                ti = 4 * j + tl

                # kv before q: wkv (1MB) lands long before the full wq
                # (4MB) during the startup window
                kv_ps = ps.tile([128, 256], F32, tag="kvps")
                for dc in range(DC):
                    nc.tensor.matmul(
                        kv_ps[:], lhsT=xt[:, dc * 128:(dc + 1) * 128],
                        rhs=wkv_sb[:, dc * 256:(dc + 1) * 256],
                        start=(dc == 0), stop=(dc == DC - 1))
                q_ps = ps.tile([128, 512], F32, tag="qps")
                for dc in range(DC):
                    nc.tensor.matmul(
                        q_ps[:], lhsT=xt[:, dc * 128:(dc + 1) * 128],
                        rhs=wq_sb[:, dc * 512:(dc + 1) * 512],
                        start=(dc == 0), stop=(dc == DC - 1))
                flush_T()  # previous chunk's transposes, now covered

                cos_t = cs_t[:, 0:NH * 64]
                sin_t = cs_t[:, NH * 64:2 * NH * 64]

                # ---- Q: drain PSUM to SBUF fast (frees the bank for the
                # next chunk's accumulation), then norm from the copy
                q_sb = np_.tile([128, 512], F32, tag="q_sb")
                nc.scalar.copy(q_sb[:], q_ps[:])
                sqq = np_.tile([128, NH], F32, tag="sqq")
                scr = np_.tile([128, 128], BF16, tag="scr")
                for n in range(NH):
                    nc.scalar.activation(
                        scr[:], q_sb[:, n * 128:(n + 1) * 128], Act.Square,
                        accum_out=sqq[:, n:n + 1])
                rq = np_.tile([128, NH], F32, tag="rq")
                nc.scalar.activation(rq[:], sqq[:], Act.Sqrt,
                                     scale=1.0 / H, bias=eps_sb[:])
                nc.vector.reciprocal(rq[:], rq[:])

                qa = np_.tile([128, 512], F32, tag="qa")
                nc.vector.tensor_mul(out=qa[:], in0=q_sb[:], in1=qsb_sb[:])
                qf = np_.tile([128, 512], F32, tag="qf")
                t1 = np_.tile([128, 256], F32, tag="t1")
                t2 = np_.tile([128, 256], F32, tag="t2")
                qa3 = qa[:].rearrange("p (n h) -> p n h", n=NH)
                qf3 = qf[:].rearrange("p (n h) -> p n h", n=NH)
                c3 = cos_t.rearrange("p (n h) -> p n h", n=NH)
                s3 = sin_t.rearrange("p (n h) -> p n h", n=NH)
                t13 = t1[:].rearrange("p (n h) -> p n h", n=NH)
                t23 = t2[:].rearrange("p (n h) -> p n h", n=NH)
                x1, x2 = qa3[:, :, 0:64], qa3[:, :, 64:128]
                nc.vector.tensor_mul(out=t13, in0=x1, in1=c3)
                nc.vector.tensor_mul(out=t23, in0=x2, in1=s3)
                nc.vector.tensor_tensor(
                    out=qf3[:, :, 0:64], in0=t13, in1=t23, op=Alu.subtract)
                nc.vector.tensor_mul(out=t13, in0=x2, in1=c3)
                nc.vector.tensor_mul(out=t23, in0=x1, in1=s3)
                nc.vector.tensor_tensor(
                    out=qf3[:, :, 64:128], in0=t13, in1=t23, op=Alu.add)
                qb = np_.tile([128, 512], FP16, tag="qb")
                for n in range(NH):
                    nc.vector.tensor_scalar_mul(
                        out=qb[:, n * 128:(n + 1) * 128],
                        in0=qf[:, n * 128:(n + 1) * 128],
                        scalar1=rq[:, n:n + 1])
                def do_q_transposes(qb=qb):
                    for n in range(NH):
                        tp = ps.tile([128, 128], FP16, tag="tp")
                        nc.tensor.transpose(
                            tp[:], qb[:, n * 128:(n + 1) * 128], ident[:])
                        nc.vector.tensor_copy(
                            out=qT[:, n * 512 + tl * 128:
                                   n * 512 + (tl + 1) * 128],
                            in_=tp[:])
                pending_T.append(do_q_transposes)

                # ---- K: rms stats, (1+ks), rope, fp16, transpose
                kv_sb = np_.tile([128, 256], F32, tag="kv_sb")
                nc.vector.tensor_copy(out=kv_sb[:], in_=kv_ps[:])
                sqk = np_.tile([128, 2], F32, tag="sqk")
                nc.scalar.activation(scr[:], kv_sb[:, 0:128], Act.Square,
                                     accum_out=sqk[:, 0:1])
                nc.scalar.activation(scr[:], kv_sb[:, 128:256], Act.Square,
                                     accum_out=sqk[:, 1:2])
                rk = np_.tile([128, 2], F32, tag="rk")
                nc.scalar.activation(rk[:], sqk[:], Act.Sqrt,
                                     scale=1.0 / H, bias=eps_sb[:])
                nc.vector.reciprocal(rk[:], rk[:])

                ka = np_.tile([128, 128], F32, tag="ka")
                nc.vector.tensor_mul(out=ka[:], in0=kv_sb[:, 0:128],
                                     in1=ksb_sb[:])
                kf = np_.tile([128, 128], F32, tag="kf")
                nc.vector.tensor_mul(out=t1[:, 0:64], in0=ka[:, 0:64],
                                     in1=cos_t[:, 0:64])
                nc.vector.tensor_mul(out=t2[:, 0:64], in0=ka[:, 64:128],
                                     in1=sin_t[:, 0:64])
                nc.vector.tensor_tensor(out=kf[:, 0:64], in0=t1[:, 0:64],
                                        in1=t2[:, 0:64], op=Alu.subtract)
                nc.vector.tensor_mul(out=t1[:, 0:64], in0=ka[:, 64:128],
                                     in1=cos_t[:, 0:64])
                nc.vector.tensor_mul(out=t2[:, 0:64], in0=ka[:, 0:64],
                                     in1=sin_t[:, 0:64])
                nc.vector.tensor_tensor(out=kf[:, 64:128], in0=t1[:, 0:64],
                                        in1=t2[:, 0:64], op=Alu.add)
                kb = np_.tile([128, 128], FP16, tag="kb")
                nc.vector.tensor_scalar_mul(out=kb[:], in0=kf[:],
                                            scalar1=rk[:, 0:1])

                def do_k_transpose(kb=kb):
                    tp = ps.tile([128, 128], FP16, tag="tp")
                    nc.tensor.transpose(tp[:], kb[:], ident[:])
                    nc.vector.tensor_copy(
                        out=kT_sb[:, ti * 128:(ti + 1) * 128], in_=tp[:])
                pending_T.append(do_k_transpose)

                # ---- V: rms only, bf16, stays [s, h]
                nc.vector.tensor_scalar_mul(
                    out=vf_sb[:, ti * 128:(ti + 1) * 128],
                    in0=kv_sb[:, 128:256], scalar1=rk[:, 1:2])

            def attn_head(j, n, qT):
                """Causal attention for local head n over t-group j; returns
                the normalized output tile outT [h, 512] (fp16, SBUF)."""
                nk = 4 * (j + 1)  # causal s-chunks for this group
                pT_tiles = []
                # running softmax denominator: pT_sum accumulated on DVE
                # (bf16), reduced over partitions by ONE ones-matmul at the
                # end instead of one per k-chunk (saves TensorE time).
                pT_sum = np_.tile([128, 512], BF16, tag="pTs", name="pTs")
                for k in range(nk):
                    lt = ps.tile([128, 512], F32, tag="lt", bufs=2)
                    dcol0 = k - 4 * j
                    lo = max(dcol0, 0) * 128  # columns left of the diagonal
                    # block are fully masked -- skip computing them
                    nc.tensor.matmul(
                        lt[:, lo:512], lhsT=kT_sb[:, k * 128:(k + 1) * 128],
                        rhs=qT[:, n * 512 + lo:(n + 1) * 512],
                        start=True, stop=True)
                    pT_k = pp.tile([128, 512], BF16, tag=f"pT{k}")
                    dcol = k - 4 * j
                    if dcol >= 0:
                        # diagonal s-chunk: mask in-block upper triangle;
                        # t-chunk columns left of it are fully masked and
                        # never computed, added, or AV-multiplied
                        nc.vector.tensor_add(
                            out=lt[:, dcol * 128:(dcol + 1) * 128],
                            in0=lt[:, dcol * 128:(dcol + 1) * 128],
                            in1=maskT_sb[:])
                        nc.scalar.activation(
                            pT_k[:, dcol * 128:512],
                            lt[:, dcol * 128:512], Act.Exp)
                    else:
                        nc.scalar.activation(pT_k[:], lt[:], Act.Exp)
                    if k == 0:
                        nc.vector.tensor_copy(out=pT_sum[:], in_=pT_k[:])
                    else:
                        lo = max(dcol, 0) * 128
                        nc.vector.tensor_add(out=pT_sum[:, lo:512],
                                             in0=pT_sum[:, lo:512],
                                             in1=pT_k[:, lo:512])
                    pT_tiles.append(pT_k)

                z = ps.tile([128, 512], F32, tag="z")
                nc.tensor.matmul(z[:], lhsT=ones_bf[:], rhs=pT_sum[:],
                                 start=True, stop=True)
                bz = np_.tile([128, 512], F32, tag="bz")
                nc.vector.reciprocal(bz[:], z[:])

                av = ps.tile([128, 512], F32, tag="av")
                for k in range(nk):
                    # k=0 is always full-width (start=True clears the whole
                    # bank); later diagonal chunks only accumulate into the
                    # unmasked column range
                    lo = max(k - 4 * j, 0) * 128
                    nc.tensor.matmul(av[:, lo:512],
                                     lhsT=vf_sb[:, k * 128:(k + 1) * 128],
                                     rhs=pT_tiles[k][:, lo:512],
                                     start=(k == 0), stop=(k == nk - 1))
                outT = op.tile([128, 512], FP16, tag=f"outT{n % 2}")
                nc.vector.tensor_mul(out=outT[:], in0=av[:], in1=bz[:])
                return outT

            def gather_pair(j, p, outT0, outT1):
                """AllGather heads 2p,2p+1 of group j in one op; returns
                per-head views [h, core, t] of the gathered buffer."""
                ag_in = dr.tile([2, 128, 512], FP16, tag=f"agin{j}_{p}")
                # scalar (hwdge) queue: faster trigger path than the sync
                # queue, which is congested with weight/x loads
                nc.scalar.dma_start(ag_in[0], outT0[:])
                nc.scalar.dma_start(ag_in[1], outT1[:])
                ag_out = dr.tile([N_CORES, 2, 128, 512], FP16,
                                 tag=f"agout{j}_{p}", addr_space="Shared")
                nc.gpsimd.collective_compute(
                    "AllGather", Alu.bypass, replica_groups=rg,
                    ins=[ag_in.rearrange("a b c -> (a b c)")],
                    outs=[ag_out.rearrange("a b c d -> (a b c d)")])
                v = ag_out.rearrange("c p h t -> p h c t")
                return v[0], v[1]

            def gather_one(j, n, outT):
                """AllGather a single head (1MB): finer tail granularity."""
                ag_in = dr.tile([128, 512], FP16, tag=f"agsin{j}_{n}")
                nc.scalar.dma_start(ag_in[:], outT[:])
                ag_out = dr.tile([N_CORES, 128, 512], FP16,
                                 tag=f"agsout{j}_{n}", addr_space="Shared")
                nc.gpsimd.collective_compute(
                    "AllGather", Alu.bypass, replica_groups=rg,
                    ins=[ag_in.rearrange("a b -> (a b)")],
                    outs=[ag_out.rearrange("a b c -> (a b c)")])
                return ag_out.rearrange("c h t -> h c t")

            def oproj_slots(ti, ags, slots, o_ps, start, stop):
                for slot in slots:
                    agt = op.tile([128, N_CORES * 128], FP16, tag="agt",
                                  bufs=3, name="agt")
                    nc.sync.dma_start(
                        agt[:].rearrange("p (a b) -> p a b", a=N_CORES),
                        ags[slot][:, :, (ti % 4) * 128:(ti % 4 + 1) * 128])
                    for c8 in range(N_CORES):
                        nhead = 4 * c8 + slot
                        nc.tensor.matmul(
                            o_ps[:],
                            lhsT=agt[:, c8 * 128:(c8 + 1) * 128],
                            rhs=wo_sb[:, nhead * 512:(nhead + 1) * 512],
                            start=(start and slot == slots[0] and c8 == 0),
                            stop=(stop and slot == slots[-1]
                                  and c8 == N_CORES - 1))

            def oproj_chunk(j, tl, ags):
                """Output projection (all 32 global heads -> local D slice)
                for t-chunk tl of group j. ags[n][c] holds core c's
                local head n = global head 4c+n."""
                ti = 4 * j + tl
                o_ps = ps.tile([128, 512], F32, tag="ops")
                oproj_slots(ti, ags, [0, 1, 2, 3], o_ps, True, True)
                o_sb = op.tile([128, 512], F32, tag="osb")
                nc.scalar.copy(o_sb[:], o_ps[:])
                nc.sync.dma_start(
                    out_d.ap()[ti * 128:(ti + 1) * 128, :], o_sb[:])

            def oproj_tail(j, ags):
                """Last group: run slots 2-3 (whose pair AllGather was
                issued FIRST in the last group) for all 4 chunks, then
                slots 0-1 as a second PSUM pass merged with a DVE add, so
                the later pair's AllGather latency is covered."""
                partials = []
                for tl in range(4):
                    o_ps = ps.tile([128, 512], F32, tag="ops")
                    oproj_slots(4 * j + tl, ags, [2, 3], o_ps, True, True)
                    o_sb = op.tile([128, 512], F32, tag="osbp", bufs=4,
                                   name="osbp")
                    nc.scalar.copy(o_sb[:], o_ps[:])
                    partials.append(o_sb)
                for tl in range(4):
                    ti = 4 * j + tl
                    o_ps = ps.tile([128, 512], F32, tag="ops")
                    oproj_slots(ti, ags, [0, 1], o_ps, True, True)
                    o_sb2 = op.tile([128, 512], F32, tag="osb2")
                    nc.vector.tensor_add(out=o_sb2[:], in0=o_ps[:],
                                         in1=partials[tl][:])
                    nc.sync.dma_start(
                        out_d.ap()[ti * 128:(ti + 1) * 128, :], o_sb2[:])

            # -------- software pipeline, interleaved at head granularity:
            # attn(j,n) ; proj(j+1,n) ; oproj(j-1,n) round-robin so no
            # engine queue gets a monolithic phase block.
            chunks = {}
            prev_ags = None
            qT_cur = np_.tile([128, NH * 512], FP16, tag="qT", name="qT")
            # interleaved preload: first x-chunk and first wq pieces lead
            def load_wq(lo, hi):
                for i in range(lo, hi):
                    nc.sync.dma_start(wq_sb[:, i * 1024:(i + 1) * 1024],
                                      wq_flat[:, i * 1024:(i + 1) * 1024])

            chunks[0] = load_chunk(0)
            for i in range(4):
                nc.sync.dma_start(wkv_sb[:, i * 2048:(i + 1) * 2048],
                                  wkv_flat[:, i * 2048:(i + 1) * 2048])
            load_wq(0, 4)
            chunks[1] = load_chunk(1)
            load_wq(4, 8)
            nc.sync.dma_start(qsb_sb[:], qsb_d.ap())
            nc.sync.dma_start(ksb_sb[:], ksb_d.ap())
            nc.sync.dma_start(maskT_sb[:], maskT_d.ap())
            # lookahead-2 chunk loads: each load is issued one proj AFTER
            # the buffer slot it needs was freed, so it never head-of-line
            # blocks the (strictly in-order) DMA queue. The remaining wq
            # pieces interleave with the chunk loads so neither stream
            # starves the other on the serial queue.
            for tl in range(4):
                proj_chunk(0, tl, qT_cur, *chunks.pop(tl))
                chunks[tl + 2] = load_chunk(tl + 2)
                if tl < 2:
                    load_wq(8 + 4 * tl, 12 + 4 * tl)
            flush_T()  # T(0,3): group-0 attention needs qT(0)/kT complete
            wo_flat = wo_d.ap().rearrange("p a b -> p (a b)")
            for j in range(NG):
                qT_next = (np_.tile([128, NH * 512], FP16, tag="qT", name="qT")
                           if j + 1 < NG else None)
                # Per head: attention first (its pair-AllGather triggers
                # early), then o-proj of the previous group and proj of the
                # next group fill TensorE under the attention latencies.
                ags = [None] * NH
                outs = [None] * NH
                if j < NG - 1:
                    for n in range(NH):
                        outs[n] = attn_head(j, n, qT_cur)
                        if n % 2 == 1:
                            ags[n - 1], ags[n] = gather_pair(
                                j, n // 2, outs[n - 1], outs[n])
                        ti = 4 * (j + 1) + n
                        proj_chunk(j + 1, n, qT_next, *chunks.pop(ti))
                        if ti + 2 < TC:
                            chunks[ti + 2] = load_chunk(ti + 2)
                        if j == 0:
                            nc.sync.dma_start(
                                wo_sb[:, n * 4096:(n + 1) * 4096],
                                wo_flat[:, n * 4096:(n + 1) * 4096])
                    # o-proj of the PREVIOUS group at the END of this block:
                    # TensorE is strict FIFO, so the agt fetches (which wait
                    # on the previous group's AllGathers) must sit behind a
                    # full block (~90us) of independent attn+proj matmuls.
                    # The last proj chunk's deferred transposes flush after
                    # the first o-proj chunk (32 matmuls of cover) -- and
                    # always before the NEXT block's attention reads qT/kT.
                    if prev_ags is not None:
                        oproj_chunk(j - 1, 0, prev_ags)
                        flush_T()
                        for n in range(1, NH):
                            oproj_chunk(j - 1, n, prev_ags)
                    else:
                        flush_T()
                else:
                    # last group: heads 2,3 first so their AllGather (needed
                    # by the tail's FIRST pass) is in flight earliest; the
                    # previous group's o-proj fills the remaining latency.
                    outs[2] = attn_head(j, 2, qT_cur)
                    outs[3] = attn_head(j, 3, qT_cur)
                    ags[2], ags[3] = gather_pair(j, 1, outs[2], outs[3])
                    outs[0] = attn_head(j, 0, qT_cur)
                    outs[1] = attn_head(j, 1, qT_cur)
                    ags[0], ags[1] = gather_pair(j, 0, outs[0], outs[1])
                    for n in range(NH):
                        oproj_chunk(j - 1, n, prev_ags)
                prev_ags = ags
                qT_cur = qT_next
            oproj_tail(NG - 1, prev_ags)

    nc.compile()
    return nc


def _get_nc():
    if "nc" not in _CACHE:
        _CACHE["nc"] = _build()
    return _CACHE["nc"]


# ---------------------------------------------------------------- host prep
def _make_in_maps(x, segment_pos, attn_mask, q_w, kv_w, o_w, q_scale, k_scale):
    x = np.asarray(x, np.float32)
    q_w = np.asarray(q_w, np.float32)
    kv_w = np.asarray(kv_w, np.float32)
    o_w = np.asarray(o_w, np.float32)
    q_scale = np.asarray(q_scale, np.float32)
    k_scale = np.asarray(k_scale, np.float32)
    pos = np.asarray(segment_pos)[0].astype(np.float32)

    x2 = x[0]  # [T, D]
    # xt[ti, p, dc, tl] = x[ti*128+tl, dc*128+p]
    xt = np.ascontiguousarray(
        x2.reshape(TC, 128, DC, 128).transpose(0, 3, 2, 1)).astype(np.float16)

    frac = 2.0 * np.arange(H // 2, dtype=np.float32) / H
    ts_ = (ROPE_BASE ** frac).astype(np.float32)
    sinu = pos[:, None] / ts_[None, :]          # [T, 64]
    csp = np.concatenate([np.tile(np.cos(sinu), (1, NH)),
                          np.tile(np.sin(sinu), (1, NH))],
                         axis=1).astype(np.float32).reshape(
        TC, 128, 2 * NH * 64)

    maskT = np.ascontiguousarray(
        np.asarray(attn_mask, np.float32)[0, :128, :128].T)

    qs_row = np.tile(1.0 + q_scale, NH)                       # [512]
    qsb = np.ascontiguousarray(
        np.broadcast_to(qs_row[None, :], (128, NH * 128))).astype(np.float32)
    ksb = np.ascontiguousarray(
        np.broadcast_to((1.0 + k_scale)[None, :], (128, 128))).astype(
            np.float32)

    in_maps = []
    for c in range(N_CORES):
        qw_c = q_w[NH * c:NH * (c + 1)]           # [4, D, H]
        # wq[p, dc, n*128+h] = qw_c[n, dc*128+p, h]
        wq = np.ascontiguousarray(
            qw_c.transpose(1, 0, 2).reshape(DC, 128, NH * H).transpose(
                1, 0, 2)).astype(np.float16)
        kv_c = kv_w[:, c]                         # [2, D, H]
        wkv = np.ascontiguousarray(
            kv_c.transpose(1, 0, 2).reshape(DC, 128, 2 * H).transpose(
                1, 0, 2)).astype(np.float16)
        # wo[h, n, dsl] = o_w[n, h, c*512 + dsl]
        wo = np.ascontiguousarray(
            o_w[:, :, DSL * c:DSL * (c + 1)].transpose(1, 0, 2)).astype(
                np.float16)
        in_maps.append({
            "xt": xt, "wq": wq, "wkv": wkv, "wo": wo,
            "csp": csp, "qsb": qsb, "ksb": ksb,
            "maskt": maskT,
        })
    return in_maps


def _execute(in_maps, trace=False):
    from concourse import bass_utils
    nc = _get_nc()
    return bass_utils.run_bass_kernel_spmd(
        nc, in_maps, core_ids=list(range(N_CORES)), trace=trace)


# ---------------------------------------------------------------- entry
def kernel(x, segment_pos, attn_mask, q_w, kv_w, o_w, q_scale, k_scale):
    in_maps = _make_in_maps(x, segment_pos, attn_mask, q_w, kv_w, o_w,
                            q_scale, k_scale)
    res = _execute(in_maps, trace=False)
    outs = [np.asarray(res.results[c]["out"]) for c in range(N_CORES)]
    full = np.concatenate(outs, axis=1).astype(np.float32)
    return full[None]

